# revision 1
# baseline (speedup 1.0000x reference)
"""Trainium2 Bass kernel for a 3-layer BiLSTM + ReLU + residual + LayerNorm.

Strategy (pure data parallel over 8 cores, 1024 batch rows per core):
  * "Transposed" layout on-chip: features on SBUF partitions, batch on the
    free dim.  Both directions fused on partitions (fwd = 0:64, bwd = 64:128)
    so every ScalarE/VectorE op runs with all 128 lanes busy.
  * Per timestep, per gate: one 128x128 block-diagonal recurrent matmul plus
    two 64-wide input-projection matmuls (col-tiled pairs) accumulate into a
    PSUM tile [g_fwd; g_bwd].  Sigmoid over a contiguous [i,f,o] PSUM span is
    a single ScalarE op; tanh(g), the c/h updates, and tanh(c) follow.
  * Layer outputs stream through DRAM as bf16 (the recurrence itself stays
    fp32); batch is processed as two interleaved chunks so the engines can
    overlap the sequential per-step dependency chain.
  * Final stage: PE transpose to natural layout + K=9 residual matmul into
    the same PSUM tile, LayerNorm stats via ScalarE accumulate outputs,
    normalize via per-partition tensor_scalar, DMA out natural-layout fp32.
"""

from contextlib import ExitStack

import numpy as np
import ml_dtypes

import concourse.bacc as bacc
import concourse.tile as tile
from concourse import mybir
from concourse.bass_utils import run_bass_kernel_spmd

F32 = mybir.dt.float32
BF16 = mybir.dt.bfloat16
AF = mybir.ActivationFunctionType
OP = mybir.AluOpType

NCORES = 8
BC = 1024               # batch rows per core
CHUNKS = 2
T = 64
H = 64
NL = 3
D2 = 2 * H              # 128
LN_EPS = 1e-5

# gate order in PyTorch weights: i, f, g, o  (rows g*H:(g+1)*H of w_ih/w_hh)
SIG_GATES = (0, 1, 3)   # i, f, o  -> sigmoid, held in one PSUM span
TANH_GATE = 2           # g        -> tanh


def _host_prep(x, w_ih, w_hh, b_ih, b_hh, w_res, b_res, ncores, bc):
    """Matmul-ready weight layouts (shared across cores) + per-core inputs."""
    x = np.asarray(x, np.float32)
    w_ih = np.asarray(w_ih, np.float32)
    w_hh = np.asarray(w_hh, np.float32)
    bias = np.asarray(b_ih, np.float32) + np.asarray(b_hh, np.float32)  # (NL,2,4H)
    w_res = np.asarray(w_res, np.float32)
    b_res = np.asarray(b_res, np.float32)
    t_len = x.shape[1]

    # Recurrent lhsT, K-major: rw[k, l, g, m] (block-diagonal over directions)
    rw = np.zeros((128, NL, 4, 128), np.float32)
    for l in range(NL):
        for g in range(4):
            gs = slice(g * H, (g + 1) * H)
            rw[0:64, l, g, 0:64] = w_hh[l, 0, gs, :].T
            rw[64:128, l, g, 64:128] = w_hh[l, 1, gs, :].T

    # Input-projection lhsT for layers 1,2 (bf16): pw[k, l-1, g, d, m]
    pw = np.zeros((128, NL - 1, 4, 2, 64), np.float32)
    for l in (1, 2):
        for g in range(4):
            gs = slice(g * H, (g + 1) * H)
            for d in range(2):
                pw[:, l - 1, g, d, :] = w_ih[l, d, gs, :].T
    pw = pw.astype(ml_dtypes.bfloat16)

    # Layer-0 projection lhsT with the bias folded into a ones-row (row 8)
    l0w = np.zeros((9, 4, 2, 64), np.float32)
    for g in range(4):
        gs = slice(g * H, (g + 1) * H)
        for d in range(2):
            l0w[0:8, g, d, :] = w_ih[0, d, gs, 0:8].T
            l0w[8, g, d, :] = bias[0, d, gs]

    # per-partition sigmoid-gate biases, layers 1,2 (fused dirs): br[p, idx]
    br = np.zeros((128, (NL - 1) * 3), np.float32)
    for l in (1, 2):
        for j, g in enumerate(SIG_GATES):
            gs = slice(g * H, (g + 1) * H)
            br[0:64, (l - 1) * 3 + j] = bias[l, 0, gs]
            br[64:128, (l - 1) * 3 + j] = bias[l, 1, gs]

    # g-gate bias per layer 1,2, per fused partition: gb[p, l-1]
    gb = np.zeros((128, NL - 1), np.float32)
    gs = slice(TANH_GATE * H, (TANH_GATE + 1) * H)
    for l in (1, 2):
        gb[0:64, l - 1] = bias[l, 0, gs]
        gb[64:128, l - 1] = bias[l, 1, gs]

    # residual rhs: wres[k, f] = w_res[f, k], row 8 = b_res
    wres = np.zeros((9, 128), np.float32)
    wres[0:8, :] = w_res.T
    wres[8, :] = b_res

    ident = np.eye(128, dtype=np.float32)

    # Per-core transposed-augmented input: xaug[k, t, b]
    xaug_cores = []
    for c in range(ncores):
        xc = x[c * bc:(c + 1) * bc]              # (bc, T, 8)
        xa = np.empty((9, t_len, bc), np.float32)
        xa[0:8] = xc.transpose(2, 1, 0)
        xa[8] = 1.0
        xaug_cores.append(xa)

    shared = dict(rw=rw, pw=pw, l0w=l0w, br=br, gb=gb, wres=wres, ident=ident)
    return shared, xaug_cores


def _emit(nc, tc, ctx, D, apply_gb, bc, t_len):
    bk = bc // CHUNKS
    fb = min(128, bk)         # final-stage block width (natural-layout rows)
    nb = bk // fb             # blocks per chunk per timestep
    strip = min(8, t_len)

    sbC = ctx.enter_context(tc.tile_pool(name="consts", bufs=1))
    sbA = ctx.enter_context(tc.tile_pool(name="workA", bufs=3))
    sbB = ctx.enter_context(tc.tile_pool(name="workB", bufs=2))
    sbS = ctx.enter_context(tc.tile_pool(name="state", bufs=1))
    sbZ = ctx.enter_context(tc.tile_pool(name="zhold", bufs=strip + 2))
    ps = ctx.enter_context(tc.tile_pool(name="ps", bufs=1, space="PSUM"))

    def const_tile(shape, dtype, key):
        t = sbC.tile(shape, dtype, name=f"c_{key}", tag=f"c_{key}")
        nc.sync.dma_start(out=t, in_=D[key])
        return t

    rw_sb = const_tile([128, NL, 4, 128], F32, "rw")
    pw_sb = const_tile([128, NL - 1, 4, 2, 64], BF16, "pw")
    l0w_sb = const_tile([9, 4, 2, 64], F32, "l0w")
    br_sb = const_tile([128, (NL - 1) * 3], F32, "br")
    gb_sb = const_tile([128, NL - 1], F32, "gb")
    wres_sb = const_tile([9, 128], F32, "wres")
    ident_sb = const_tile([128, 128], F32, "ident")
    gamma_sb = beta_sb = None
    if apply_gb:
        gamma_sb = const_tile([fb, 128], F32, "gammab")
        beta_sb = const_tile([fb, 128], F32, "betab")
    ones_sb = sbC.tile([1, bk], F32)
    nc.vector.memset(ones_sb, 1.0)
    eps_sb = sbC.tile([128, 1], F32)
    nc.vector.memset(eps_sb, LN_EPS)

    O = [D[f"o{i}"] for i in range(NL)]
    xaug = D["xaug"]
    out_d = D["out"]

    h_prev = [None] * CHUNKS
    c_st = [None] * CHUNKS

    def issue_inp(cc, l, k):
        # issued ahead of the consuming step so input reads enter the DMA
        # queue before the chain-tail output writes (no head-of-line block)
        c0 = cc * bk
        cols = slice(c0, c0 + bk)
        rt = t_len - 1 - k
        if l == 0:
            inp_f = sbA.tile([9, bk], F32, tag=f"inf{cc}", bufs=4, name="inp_f")
            nc.sync.dma_start(out=inp_f, in_=xaug[:, k, cols])
            inp_b = sbA.tile([9, bk], F32, tag=f"inb{cc}", bufs=4, name="inp_b")
            nc.sync.dma_start(out=inp_b, in_=xaug[:, rt, cols])
        else:
            inp_f = sbA.tile([128, bk], BF16, tag=f"inf{cc}", bufs=4, name="inp_f")
            nc.sync.dma_start(out=inp_f, in_=O[l - 1][:, k, cols])
            inp_b = sbA.tile([128, bk], BF16, tag=f"inb{cc}", bufs=4, name="inp_b")
            nc.sync.dma_start(out=inp_b, in_=O[l - 1][:, rt, cols])
        return inp_f, inp_b

    def lstm_step(cc, l, k, inp_f, inp_b):
        c0 = cc * bk
        cols = slice(c0, c0 + bk)
        rt = t_len - 1 - k

        P_ifo = ps.tile([128, 3, bk], F32, tag=f"pifo{cc}")
        P_g = ps.tile([128, bk], F32, tag=f"pg{cc}")

        def gate_mms(out_ap, g, j):
            calls = []  # (out, lhsT, rhs, tile_position, partition_range)
            w = l0w_sb if l == 0 else pw_sb
            wf = w[:, g, 0, :] if l == 0 else w[:, l - 1, g, 0, :]
            wb = w[:, g, 1, :] if l == 0 else w[:, l - 1, g, 1, :]
            calls.append((out_ap[0:64, :], wf, inp_f, (0, 0), (0, 64)))
            calls.append((out_ap[64:128, :], wb, inp_b, (0, 64), (64, 128)))
            if k > 0:
                calls.append((out_ap, rw_sb[:, l, g, :], h_prev[cc], None,
                              (0, 128)))
            n = len(calls)
            for i, (o, lh, rh, tp, rng) in enumerate(calls):
                # start: this call's partitions not all covered by earlier calls
                covered = set()
                for _, _, _, _, r in calls[:i]:
                    covered.update(range(*r))
                start = not set(range(*rng)).issubset(covered)
                # stop: no later call touches this call's partitions
                stop = not any(max(rng[0], r[0]) < min(rng[1], r[1])
                               for _, _, _, _, r in calls[i + 1:])
                # skip_group_check: the executing-sim group checker
                # mis-addresses partition-based PSUM offsets (tensor rows
                # != 16KB); data semantics are still simulated exactly.
                nc.tensor.matmul(o, lh, rh, start=start, stop=stop,
                                 tile_position=tp, skip_group_check=True)

        for j, g in enumerate(SIG_GATES):
            gate_mms(P_ifo[:, j, :], g, j)
        gate_mms(P_g, TANH_GATE, None)

        S_ifo = sbB.tile([128, 3, bk], F32, tag=f"sifo{cc}", bufs=3)
        S_g = sbB.tile([128, bk], F32, tag=f"sg{cc}")

        def sig(j):
            if l > 0:
                idx = (l - 1) * 3 + j
                nc.scalar.activation(out=S_ifo[:, j, :], in_=P_ifo[:, j, :],
                                     func=AF.Sigmoid,
                                     bias=br_sb[:, idx:idx + 1])
            else:
                nc.scalar.activation(out=S_ifo[:, j, :], in_=P_ifo[:, j, :],
                                     func=AF.Sigmoid)

        sig(0)                                                    # i
        if l > 0:
            nc.scalar.activation(out=S_g, in_=P_g, func=AF.Tanh,
                                 bias=gb_sb[:, l - 1:l])
        else:
            nc.scalar.activation(out=S_g, in_=P_g, func=AF.Tanh)
        sig(1)                                                    # f
        sig(2)                                                    # o

        if k == 0:
            c = sbS.tile([128, bk], F32, tag=f"c{cc}")
            c_st[cc] = c
            nc.vector.tensor_mul(c, S_ifo[:, 0, :], S_g)          # c = i*g
        else:
            c = c_st[cc]
            tmp = sbB.tile([128, bk], F32, tag=f"tmp{cc}")
            nc.gpsimd.tensor_mul(tmp, S_ifo[:, 0, :], S_g)        # i*g (POOL)
            nc.vector.tensor_mul(c, S_ifo[:, 1, :], c)            # f*c
            nc.vector.tensor_add(c, c, tmp)
        return S_ifo, c

    def lstm_step_ph2(cc, l, k, S_ifo, c):
        # second phase emitted after the other chunk's phase 1 so the
        # ScalarE FIFO never head-of-line blocks on tanh(c) while the other
        # chunk's (ready) sigmoid sits behind it
        c0 = cc * bk
        cols = slice(c0, c0 + bk)
        rt = t_len - 1 - k
        Tc = sbB.tile([128, bk], F32, tag=f"tc{cc}")
        nc.scalar.activation(out=Tc, in_=c, func=AF.Tanh)
        h = sbA.tile([128, bk], F32, tag=f"h{cc}")
        nc.vector.tensor_mul(h, S_ifo[:, 2, :], Tc)               # h = o*tanh(c)
        h_prev[cc] = h

        # cast + store time-ordered halves: fwd half at t=k, bwd half at t=rt
        h_bf = sbA.tile([128, bk], BF16, tag=f"hbf{cc}")
        nc.gpsimd.tensor_copy(out=h_bf, in_=h)
        nc.sync.dma_start(out=O[l][0:64, k, cols], in_=h_bf[0:64, :])
        nc.sync.dma_start(out=O[l][64:128, rt, cols], in_=h_bf[64:128, :])

    PF = min(2, t_len - 1)
    for l in range(NL):
        pend = {}
        for kk in range(PF):
            for cc in range(CHUNKS):
                pend[(cc, kk)] = issue_inp(cc, l, kk)
        for k in range(t_len):
            ph1 = {}
            for cc in range(CHUNKS):
                if k + PF < t_len:
                    pend[(cc, k + PF)] = issue_inp(cc, l, k + PF)
                inp_f, inp_b = pend.pop((cc, k))
                ph1[cc] = lstm_step(cc, l, k, inp_f, inp_b)
            for cc in range(CHUNKS):
                S_ifo, c = ph1[cc]
                lstm_step_ph2(cc, l, k, S_ifo, c)

    # ---- final stage: relu + residual + LayerNorm + transpose to natural ----
    sums = [sbS.tile([fb, nb, t_len], F32, tag=f"sums{cc}", name=f"sums{cc}")
            for cc in range(CHUNKS)]
    sumsq = [sbS.tile([fb, nb, t_len], F32, tag=f"sumsq{cc}", name=f"sumsq{cc}")
             for cc in range(CHUNKS)]

    def issue_fin(cc, t):
        c0 = cc * bk
        cols = slice(c0, c0 + bk)
        o2t = sbA.tile([128, bk], BF16, tag=f"inf{cc}", bufs=4, name="o2t")
        nc.sync.dma_start(out=o2t, in_=O[NL - 1][:, t, cols])
        xt = sbA.tile([9, bk], F32, tag=f"inb{cc}", bufs=4, name="xt")
        nc.sync.dma_start(out=xt, in_=xaug[:, t, cols])
        return o2t, xt

    def final_t(cc, t, zs, o2t, xt):
        c0 = cc * bk
        cols = slice(c0, c0 + bk)
        relu4 = sbB.tile([128, bk], F32, tag=f"relu{cc}")
        nc.gpsimd.tensor_scalar_max(relu4, o2t, 0.0)
        # one accumulation group for the whole bank: transpose overwrites its
        # quarter (pending-zero from the single start), residual accumulates
        psZ = ps.tile([fb, nb, 128], F32, tag=f"pg{cc}")
        for bi in range(nb):
            bs = slice(bi * fb, (bi + 1) * fb)
            nc.tensor.matmul(psZ[:, bi, :], relu4[:, bs], ident_sb,
                             is_transpose=True, start=(bi == 0), stop=False,
                             skip_group_check=True)
            nc.tensor.matmul(psZ[:, bi, :], xt[:, bs], wres_sb,
                             start=False, stop=(bi == nb - 1),
                             skip_group_check=True)
        z = sbZ.tile([fb, nb, 128], F32, tag=f"z{cc}")
        z2 = sbB.tile([fb, nb, 128], F32, tag=f"z2{cc}")
        for bi in range(nb):
            nc.scalar.activation(out=z[:, bi, :], in_=psZ[:, bi, :],
                                 func=AF.Identity,
                                 accum_out=sums[cc][:, bi, t:t + 1])
            nc.scalar.activation(out=z2[:, bi, :], in_=psZ[:, bi, :],
                                 func=AF.Square,
                                 accum_out=sumsq[cc][:, bi, t:t + 1])
        zs.append((t, z))

    def final_strip_norm(cc, t0, zs):
        c0 = cc * bk
        ss = slice(t0, t0 + strip)
        mu = sbB.tile([fb, nb, strip], F32, tag=f"mu{cc}")
        nc.vector.tensor_scalar_mul(mu, sums[cc][:, :, ss], 1.0 / D2)
        var = sbB.tile([fb, nb, strip], F32, tag=f"var{cc}")
        nc.vector.tensor_scalar_mul(var, sumsq[cc][:, :, ss], 1.0 / D2)
        mu2 = sbB.tile([fb, nb, strip], F32, tag=f"mu2{cc}")
        nc.vector.tensor_mul(mu2, mu, mu)
        nc.vector.tensor_sub(var, var, mu2)
        sd = sbB.tile([fb, nb, strip], F32, tag=f"sd{cc}")
        nc.scalar.activation(out=sd, in_=var, func=AF.Sqrt,
                             bias=eps_sb[0:fb, 0:1])
        rstd = sbB.tile([fb, nb, strip], F32, tag=f"rstd{cc}")
        nc.vector.reciprocal(rstd, sd)
        nmr = sbB.tile([fb, nb, strip], F32, tag=f"nmr{cc}")
        nc.vector.scalar_tensor_tensor(nmr, mu, -1.0, rstd,
                                       op0=OP.mult, op1=OP.mult)
        for (t, z) in zs:
            ti = t - t0
            for bi in range(nb):
                on = sbA.tile([fb, 128], F32, tag=f"on{cc}")
                nc.vector.tensor_scalar(on, z[:, bi, :],
                                        rstd[:, bi, ti:ti + 1],
                                        nmr[:, bi, ti:ti + 1],
                                        op0=OP.mult, op1=OP.add)
                if apply_gb:
                    nc.vector.tensor_mul(on, on, gamma_sb)
                    nc.vector.tensor_add(on, on, beta_sb)
                b0 = c0 + bi * fb
                nc.sync.dma_start(out=out_d[b0:b0 + fb, t, :], in_=on)

    fpend = {}
    for tt in range(PF):
        for cc in range(CHUNKS):
            fpend[(cc, tt)] = issue_fin(cc, tt)
    for t0 in range(0, t_len, strip):
        zstrip = [[] for _ in range(CHUNKS)]
        for t in range(t0, t0 + strip):
            for cc in range(CHUNKS):
                if t + PF < t_len:
                    fpend[(cc, t + PF)] = issue_fin(cc, t + PF)
                o2t, xt = fpend.pop((cc, t))
                final_t(cc, t, zstrip[cc], o2t, xt)
        for cc in range(CHUNKS):
            final_strip_norm(cc, t0, zstrip[cc])


def build(apply_gb=False, bc=BC, t_len=T, num_devices=NCORES):
    nc = bacc.Bacc("TRN2", target_bir_lowering=False, debug=False,
                   num_devices=num_devices)
    fb = min(128, bc // CHUNKS)
    D = {}

    def inp(name, shape, dtype=F32):
        D[name] = nc.dram_tensor(name, shape, dtype, kind="ExternalInput").ap()

    inp("xaug", [9, t_len, bc])
    inp("rw", [128, NL, 4, 128])
    inp("pw", [128, NL - 1, 4, 2, 64], BF16)
    inp("l0w", [9, 4, 2, 64])
    inp("br", [128, (NL - 1) * 3])
    inp("gb", [128, NL - 1])
    inp("wres", [9, 128])
    inp("ident", [128, 128])
    if apply_gb:
        inp("gammab", [fb, 128])
        inp("betab", [fb, 128])
    for i in range(NL):
        D[f"o{i}"] = nc.dram_tensor(f"o{i}", [128, t_len, bc], BF16).ap()
    D["out"] = nc.dram_tensor("out", [bc, t_len, 128], F32,
                              kind="ExternalOutput").ap()

    with tile.TileContext(nc) as tc:
        with ExitStack() as ctx:
            _emit(nc, tc, ctx, D, apply_gb, bc, t_len)
    nc.compile()
    return nc


_BUILD_CACHE = {}


def kernel(x, w_ih, w_hh, b_ih, b_hh, w_res, b_res, ln_gamma, ln_beta):
    ln_gamma = np.asarray(ln_gamma, np.float32)
    ln_beta = np.asarray(ln_beta, np.float32)
    apply_gb = not (np.all(ln_gamma == 1.0) and np.all(ln_beta == 0.0))

    shared, xaug_cores = _host_prep(x, w_ih, w_hh, b_ih, b_hh, w_res, b_res,
                                    NCORES, BC)
    if apply_gb not in _BUILD_CACHE:
        _BUILD_CACHE[apply_gb] = build(apply_gb)
    nc = _BUILD_CACHE[apply_gb]

    in_maps = []
    for c in range(NCORES):
        m = dict(shared)
        m["xaug"] = xaug_cores[c]
        if apply_gb:
            fb = min(128, BC // CHUNKS)
            m["gammab"] = np.ascontiguousarray(
                np.broadcast_to(ln_gamma, (fb, 128)).astype(np.float32))
            m["betab"] = np.ascontiguousarray(
                np.broadcast_to(ln_beta, (fb, 128)).astype(np.float32))
        in_maps.append(m)

    res = run_bass_kernel_spmd(nc, in_maps, core_ids=list(range(NCORES)))
    out = np.concatenate([res.results[c]["out"] for c in range(NCORES)], axis=0)
    return np.ascontiguousarray(out.astype(np.float32))



# revision 8
# speedup vs baseline: 1.7424x; 1.7424x over previous
"""Trainium2 Bass kernel for a 3-layer BiLSTM + ReLU + residual + LayerNorm.

Strategy (pure data parallel over 8 cores, 1024 batch rows per core):
  * "Transposed" layout on-chip: features on SBUF partitions, batch on the
    free dim.  Both directions fused on partitions (fwd = 0:64, bwd = 64:128)
    so every engine op runs with all 128 lanes busy.
  * All matmuls in bf16 (4x cheaper per row than fp32 on the PE): per
    timestep, per gate, a col-tiled input-projection pair plus one 128x128
    block-diagonal recurrent matmul accumulate into one PSUM bank; the four
    gates share a [128, 4, bk] PSUM tensor ordered (i, f, o, g).
  * Layer 0 folds both time directions AND the biases into a single K=18
    augmented input ([x_t; 1; x_rt; 1]), so its activations need no bias:
    one sigmoid over the (i,f,o) span + one tanh for g.
  * Layers 1-2 keep per-gate activations with per-partition bias.
  * Elementwise rebalance: i*g and h=o*tanh(c) on DVE in bf16, f*c on the
    Pool engine, c accumulates in fp32 on DVE.
  * Final stage: bf16 PE transpose to natural layout + K=9 residual matmul
    into one PSUM bank, LayerNorm stats via one bn_stats + four bn_aggr on
    DVE, rstd via the approx reciprocal, normalization on Pool directly from
    PSUM, DMA out natural-layout fp32.
  * Hot-loop input DMAs issue from the Pool queue (cheap dispatch), output
    stores from the SP queue.
"""

from contextlib import ExitStack

import numpy as np
import ml_dtypes

import concourse.bacc as bacc
import concourse.tile as tile
from concourse import mybir
from concourse.bass_utils import run_bass_kernel_spmd

F32 = mybir.dt.float32
BF16 = mybir.dt.bfloat16
AF = mybir.ActivationFunctionType
OP = mybir.AluOpType

NCORES = 8
BC = 1024               # batch rows per core
CHUNKS = 2
T = 64
H = 64
NL = 3
D2 = 2 * H              # 128
LN_EPS = 1e-5

# gate order in PyTorch weights: i, f, g, o (rows g*H:(g+1)*H of w_ih/w_hh)
# on-chip slot order: i, f, o, g  (so i,f,o sigmoids are one contiguous span)
PT2SLOT = {0: 0, 1: 1, 2: 3, 3: 2}   # pytorch gate idx -> PSUM slot
SLOT_G = 3


def _host_prep(x, w_ih, w_hh, b_ih, b_hh, w_res, b_res, ncores, bc):
    """Matmul-ready bf16 weight layouts (shared across cores) + per-core
    inputs."""
    x = np.asarray(x, np.float32)
    w_ih = np.asarray(w_ih, np.float32)
    w_hh = np.asarray(w_hh, np.float32)
    bias = np.asarray(b_ih, np.float32) + np.asarray(b_hh, np.float32)  # (NL,2,4H)
    w_res = np.asarray(w_res, np.float32)
    b_res = np.asarray(b_res, np.float32)
    t_len = x.shape[1]

    # Recurrent lhsT, K-major: rw[k, l, slot, m] (block-diagonal over dirs)
    rw = np.zeros((128, NL, 4, 128), np.float32)
    for l in range(NL):
        for g in range(4):
            s = PT2SLOT[g]
            gs = slice(g * H, (g + 1) * H)
            rw[0:64, l, s, 0:64] = w_hh[l, 0, gs, :].T
            rw[64:128, l, s, 64:128] = w_hh[l, 1, gs, :].T
    rw = rw.astype(ml_dtypes.bfloat16)

    # Input-projection lhsT for layers 1,2 (bf16): pw[k, l-1, slot, d, m]
    pw = np.zeros((128, NL - 1, 4, 2, 64), np.float32)
    for l in (1, 2):
        for g in range(4):
            s = PT2SLOT[g]
            gs = slice(g * H, (g + 1) * H)
            for d in range(2):
                pw[:, l - 1, s, d, :] = w_ih[l, d, gs, :].T
    pw = pw.astype(ml_dtypes.bfloat16)

    # Layer-0 fused fwd+bwd projection lhsT with biases on the ones-rows:
    # K rows 0:8 = x_t, row 8 = 1, rows 9:17 = x_rt, row 17 = 1.
    l0w = np.zeros((18, 4, 128), np.float32)
    for g in range(4):
        s = PT2SLOT[g]
        gs = slice(g * H, (g + 1) * H)
        l0w[0:8, s, 0:64] = w_ih[0, 0, gs, 0:8].T
        l0w[8, s, 0:64] = bias[0, 0, gs]
        l0w[9:17, s, 64:128] = w_ih[0, 1, gs, 0:8].T
        l0w[17, s, 64:128] = bias[0, 1, gs]
    l0w = l0w.astype(ml_dtypes.bfloat16)

    # per-partition gate biases, layers 1,2 (fused dirs): br[p, (l-1)*4+slot]
    br = np.zeros((128, (NL - 1) * 4), np.float32)
    for l in (1, 2):
        for g in range(4):
            s = PT2SLOT[g]
            gs = slice(g * H, (g + 1) * H)
            br[0:64, (l - 1) * 4 + s] = bias[l, 0, gs]
            br[64:128, (l - 1) * 4 + s] = bias[l, 1, gs]

    # residual rhs: wres[k, f] = w_res[f, k], row 8 = b_res
    wres = np.zeros((9, 128), np.float32)
    wres[0:8, :] = w_res.T
    wres[8, :] = b_res
    wres = wres.astype(ml_dtypes.bfloat16)

    ident = np.eye(128, dtype=np.float32)

    # Per-core transposed-augmented input: xaug[k, t, b] with both time
    # directions stacked: rows 0:8 = x_t, 8 = 1, 9:17 = x_{T-1-t}, 17 = 1
    xaug_cores = []
    for c in range(ncores):
        xc = x[c * bc:(c + 1) * bc]              # (bc, T, 8)
        xa = np.empty((18, t_len, bc), np.float32)
        xa[0:8] = xc.transpose(2, 1, 0)
        xa[8] = 1.0
        xa[9:17] = xc[:, ::-1].transpose(2, 1, 0)
        xa[17] = 1.0
        xaug_cores.append(xa.astype(ml_dtypes.bfloat16))

    shared = dict(rw=rw, pw=pw, l0w=l0w, br=br, wres=wres, ident=ident)
    return shared, xaug_cores


def _emit(nc, tc, ctx, D, apply_gb, bc, t_len):
    bk = bc // CHUNKS

    sbC = ctx.enter_context(tc.tile_pool(name="consts", bufs=1))
    sbA = ctx.enter_context(tc.tile_pool(name="workA", bufs=3))
    sbB = ctx.enter_context(tc.tile_pool(name="workB", bufs=2))
    sbS = ctx.enter_context(tc.tile_pool(name="state", bufs=1))
    ps = ctx.enter_context(tc.tile_pool(name="ps", bufs=1, space="PSUM"))

    def const_tile(shape, dtype, key):
        t = sbC.tile(shape, dtype, name=f"c_{key}", tag=f"c_{key}")
        nc.sync.dma_start(out=t, in_=D[key])
        return t

    rw_sb = const_tile([128, NL, 4, 128], BF16, "rw")
    pw_sb = const_tile([128, NL - 1, 4, 2, 64], BF16, "pw")
    l0w_sb = const_tile([18, 4, 128], BF16, "l0w")
    br_sb = const_tile([128, (NL - 1) * 4], F32, "br")
    wres_sb = const_tile([9, 128], BF16, "wres")
    ident_sb = const_tile([128, 128], F32, "ident")
    gamma_sb = beta_sb = None
    if apply_gb:
        gamma_sb = const_tile([128, 128], F32, "gammab")
        beta_sb = const_tile([128, 128], F32, "betab")
    eps_sb = sbC.tile([128, 1], F32)
    nc.vector.memset(eps_sb, LN_EPS)

    O = [D[f"o{i}"] for i in range(NL)]
    xaug = D["xaug"]
    out_d = D["out"]

    h_prev = [None] * CHUNKS
    c_st = [None] * CHUNKS

    def issue_inp(cc, l, k):
        c0 = cc * bk
        cols = slice(c0, c0 + bk)
        rt = t_len - 1 - k
        if l == 0:
            xin = sbA.tile([18, bk], BF16, tag=f"inf{cc}", bufs=4, name="xin")
            nc.gpsimd.dma_start(out=xin, in_=xaug[:, k, cols])
            return (xin,)
        inp_f = sbA.tile([128, bk], BF16, tag=f"inf{cc}", bufs=4, name="inp_f")
        nc.gpsimd.dma_start(out=inp_f, in_=O[l - 1][:, k, cols])
        inp_b = sbA.tile([128, bk], BF16, tag=f"inb{cc}", bufs=4, name="inp_b")
        nc.gpsimd.dma_start(out=inp_b, in_=O[l - 1][:, rt, cols])
        return (inp_f, inp_b)

    def lstm_step(cc, l, k, inps):
        G = ps.tile([128, 4, bk], F32, tag=f"g{cc}")

        for s in range(4):
            if l == 0:
                (xin,) = inps
                # fused fwd+bwd proj with bias rows, K=18
                nc.tensor.matmul(G[:, s, :], l0w_sb[:, s, :], xin,
                                 start=True, stop=(k == 0),
                                 skip_group_check=True)
            else:
                inp_f, inp_b = inps
                calls = [
                    (G[0:64, s, :], pw_sb[:, l - 1, s, 0, :], inp_f,
                     (0, 0), (0, 64)),
                    (G[64:128, s, :], pw_sb[:, l - 1, s, 1, :], inp_b,
                     (0, 64), (64, 128)),
                ]
                if k > 0:
                    calls.append((G[:, s, :], rw_sb[:, l, s, :], h_prev[cc],
                                  None, (0, 128)))
                n = len(calls)
                for i, (o, lh, rh, tp, rng) in enumerate(calls):
                    covered = set()
                    for _, _, _, _, r in calls[:i]:
                        covered.update(range(*r))
                    start = not set(range(*rng)).issubset(covered)
                    stop = not any(max(rng[0], r[0]) < min(rng[1], r[1])
                                   for _, _, _, _, r in calls[i + 1:])
                    # skip_group_check: the executing-sim group checker
                    # mis-addresses partition-based PSUM offsets (tensor rows
                    # != 16KB); data semantics are still simulated exactly.
                    nc.tensor.matmul(o, lh, rh, start=start, stop=stop,
                                     tile_position=tp, skip_group_check=True)
            if l == 0 and k > 0:
                nc.tensor.matmul(G[:, s, :], rw_sb[:, 0, s, :], h_prev[cc],
                                 start=False, stop=True,
                                 skip_group_check=True)

        S = sbB.tile([128, 4, bk], BF16, tag=f"s{cc}", bufs=3)
        if l == 0:
            # biases folded into the projection: two bias-free activations
            nc.scalar.activation(out=S[:, SLOT_G, :], in_=G[:, SLOT_G, :],
                                 func=AF.Tanh)
            nc.scalar.activation(out=S[:, 0:3, :], in_=G[:, 0:3, :],
                                 func=AF.Sigmoid)
        else:
            b0 = (l - 1) * 4
            for s, fn in ((0, AF.Sigmoid), (SLOT_G, AF.Tanh),
                          (1, AF.Sigmoid), (2, AF.Sigmoid)):
                nc.scalar.activation(out=S[:, s, :], in_=G[:, s, :], func=fn,
                                     bias=br_sb[:, b0 + s:b0 + s + 1])

        if k == 0:
            c = sbS.tile([128, bk], F32, tag=f"c{cc}")
            c_st[cc] = c
            nc.vector.tensor_mul(c, S[:, 0, :], S[:, SLOT_G, :])   # c = i*g
        else:
            c = c_st[cc]
            u = sbB.tile([128, bk], BF16, tag=f"u{cc}")
            nc.vector.tensor_mul(u, S[:, 0, :], S[:, SLOT_G, :])   # i*g (DVE)
            nc.gpsimd.tensor_mul(c, S[:, 1, :], c)                 # f*c (POOL)
            nc.vector.tensor_add(c, c, u)
        return S, c

    def lstm_step_ph2(cc, l, k, S, c):
        # second phase emitted after the other chunk's phase 1 so the
        # ScalarE FIFO never head-of-line blocks on tanh(c) while the other
        # chunk's (ready) sigmoid sits behind it
        c0 = cc * bk
        cols = slice(c0, c0 + bk)
        rt = t_len - 1 - k
        Tc = sbB.tile([128, bk], BF16, tag=f"tc{cc}")
        nc.scalar.activation(out=Tc, in_=c, func=AF.Tanh)
        h = sbA.tile([128, bk], BF16, tag=f"h{cc}")
        nc.vector.tensor_mul(h, S[:, 2, :], Tc)                    # h = o*tanh(c)
        h_prev[cc] = h

        # store time-ordered halves: fwd half at t=k, bwd half at t=rt
        nc.sync.dma_start(out=O[l][0:64, k, cols], in_=h[0:64, :])
        nc.sync.dma_start(out=O[l][64:128, rt, cols], in_=h[64:128, :])

    PF = min(2, t_len - 1)
    for l in range(NL):
        pend = {}
        for kk in range(PF):
            for cc in range(CHUNKS):
                pend[(cc, kk)] = issue_inp(cc, l, kk)
        for k in range(t_len):
            ph1 = {}
            for cc in range(CHUNKS):
                if k + PF < t_len:
                    pend[(cc, k + PF)] = issue_inp(cc, l, k + PF)
                ph1[cc] = lstm_step(cc, l, k, pend.pop((cc, k)))
            for cc in range(CHUNKS):
                S, c = ph1[cc]
                lstm_step_ph2(cc, l, k, S, c)

    # ---- final stage: relu + residual + LayerNorm + transpose to natural ----
    def issue_fin(cc, t):
        c0 = cc * bk
        cols = slice(c0, c0 + bk)
        o2t = sbA.tile([128, bk], BF16, tag=f"inf{cc}", bufs=4, name="o2t")
        nc.gpsimd.dma_start(out=o2t, in_=O[NL - 1][:, t, cols])
        xt = sbA.tile([9, bk], BF16, tag=f"inb{cc}", bufs=4, name="xt")
        nc.gpsimd.dma_start(out=xt, in_=xaug[0:9, t, cols])
        return o2t, xt

    def final_t(cc, t, o2t, xt):
        c0 = cc * bk
        relu4 = sbB.tile([128, bk], F32, tag=f"relu{cc}")
        nc.vector.tensor_scalar_max(relu4, o2t, 0.0)
        # one accumulation group for the whole bank: transpose overwrites its
        # quarter (pending-zero from the single start), residual accumulates
        psZ = ps.tile([128, 4, 128], F32, tag=f"g{cc}")
        for bi in range(4):
            bs = slice(bi * 128, (bi + 1) * 128)
            nc.tensor.matmul(psZ[:, bi, :], relu4[:, bs], ident_sb,
                             is_transpose=True, start=(bi == 0), stop=False,
                             skip_group_check=True)
            nc.tensor.matmul(psZ[:, bi, :], xt[:, bs], wres_sb,
                             start=False, stop=(bi == 3),
                             skip_group_check=True)
        # LayerNorm stats over the feature dim (free within each block)
        bnst = sbB.tile([128, 4, 6], F32, tag=f"bn{cc}")
        muvar = sbB.tile([128, 4, 2], F32, tag=f"mv{cc}")
        for bi in range(4):
            nc.vector.bn_stats(bnst[:, bi, :], psZ[:, bi, :])
            nc.vector.bn_aggr(muvar[:, bi, :], bnst[:, bi, :])
        sd = sbB.tile([128, 4, 1], F32, tag=f"sd{cc}")
        nc.scalar.activation(out=sd, in_=muvar[:, :, 1:2], func=AF.Sqrt,
                             bias=eps_sb)
        rstd = sbB.tile([128, 4, 1], F32, tag=f"rs{cc}")
        nc.vector.reciprocal_approx_fast(rstd, sd)
        nmr = sbB.tile([128, 4, 1], F32, tag=f"nm{cc}")
        nc.vector.scalar_tensor_tensor(nmr, muvar[:, :, 0:1], -1.0, rstd,
                                       op0=OP.mult, op1=OP.mult)
        for bi in range(4):
            onat = sbA.tile([128, 128], F32, tag=f"on{cc}", bufs=4)
            nc.vector.tensor_scalar(onat, psZ[:, bi, :], rstd[:, bi, :],
                                    nmr[:, bi, :], op0=OP.mult, op1=OP.add)
            if apply_gb:
                nc.gpsimd.tensor_mul(onat, onat, gamma_sb)
                nc.gpsimd.tensor_add(onat, onat, beta_sb)
            b0 = c0 + bi * 128
            nc.sync.dma_start(out=out_d[b0:b0 + 128, t, :], in_=onat)

    fpend = {}
    for tt in range(PF):
        for cc in range(CHUNKS):
            fpend[(cc, tt)] = issue_fin(cc, tt)
    for t in range(t_len):
        for cc in range(CHUNKS):
            if t + PF < t_len:
                fpend[(cc, t + PF)] = issue_fin(cc, t + PF)
            o2t, xt = fpend.pop((cc, t))
            final_t(cc, t, o2t, xt)


def build(apply_gb=False, bc=BC, t_len=T, num_devices=NCORES):
    nc = bacc.Bacc("TRN2", target_bir_lowering=False, debug=False,
                   num_devices=num_devices)
    D = {}

    def inp(name, shape, dtype=F32):
        D[name] = nc.dram_tensor(name, shape, dtype, kind="ExternalInput").ap()

    inp("xaug", [18, t_len, bc], BF16)
    inp("rw", [128, NL, 4, 128], BF16)
    inp("pw", [128, NL - 1, 4, 2, 64], BF16)
    inp("l0w", [18, 4, 128], BF16)
    inp("br", [128, (NL - 1) * 4])
    inp("wres", [9, 128], BF16)
    inp("ident", [128, 128])
    if apply_gb:
        inp("gammab", [128, 128])
        inp("betab", [128, 128])
    for i in range(NL):
        D[f"o{i}"] = nc.dram_tensor(f"o{i}", [128, t_len, bc], BF16).ap()
    D["out"] = nc.dram_tensor("out", [bc, t_len, 128], F32,
                              kind="ExternalOutput").ap()

    with tile.TileContext(nc) as tc:
        with ExitStack() as ctx:
            _emit(nc, tc, ctx, D, apply_gb, bc, t_len)
    nc.compile()
    return nc


_BUILD_CACHE = {}


def make_in_maps(inputs, apply_gb):
    ln_gamma = np.asarray(inputs["ln_gamma"], np.float32)
    ln_beta = np.asarray(inputs["ln_beta"], np.float32)
    shared, xaug_cores = _host_prep(
        inputs["x"], inputs["w_ih"], inputs["w_hh"], inputs["b_ih"],
        inputs["b_hh"], inputs["w_res"], inputs["b_res"], NCORES, BC)
    in_maps = []
    for c in range(NCORES):
        m = dict(shared)
        m["xaug"] = xaug_cores[c]
        if apply_gb:
            m["gammab"] = np.ascontiguousarray(
                np.broadcast_to(ln_gamma, (128, 128)).astype(np.float32))
            m["betab"] = np.ascontiguousarray(
                np.broadcast_to(ln_beta, (128, 128)).astype(np.float32))
        in_maps.append(m)
    return in_maps


def kernel(x, w_ih, w_hh, b_ih, b_hh, w_res, b_res, ln_gamma, ln_beta):
    ln_gamma = np.asarray(ln_gamma, np.float32)
    ln_beta = np.asarray(ln_beta, np.float32)
    apply_gb = not (np.all(ln_gamma == 1.0) and np.all(ln_beta == 0.0))

    if apply_gb not in _BUILD_CACHE:
        _BUILD_CACHE[apply_gb] = build(apply_gb)
    nc = _BUILD_CACHE[apply_gb]

    inputs = dict(x=x, w_ih=w_ih, w_hh=w_hh, b_ih=b_ih, b_hh=b_hh,
                  w_res=w_res, b_res=b_res, ln_gamma=ln_gamma, ln_beta=ln_beta)
    in_maps = make_in_maps(inputs, apply_gb)

    res = run_bass_kernel_spmd(nc, in_maps, core_ids=list(range(NCORES)))
    out = np.concatenate([res.results[c]["out"] for c in range(NCORES)], axis=0)
    return np.ascontiguousarray(out.astype(np.float32))


# revision 12
# speedup vs baseline: 1.9488x; 1.1185x over previous
"""Trainium2 Bass kernel for a 3-layer BiLSTM + ReLU + residual + LayerNorm.

Strategy (pure data parallel over 8 cores, 1024 batch rows per core):
  * "Transposed" layout on-chip: features on SBUF partitions, batch on the
    free dim.  Both directions fused on partitions (fwd = 0:64, bwd = 64:128)
    so every engine op runs with all 128 lanes busy.
  * All matmuls in bf16 (4x cheaper per row than fp32 on the PE): per
    timestep, per gate, a col-tiled input-projection pair plus one 128x128
    block-diagonal recurrent matmul accumulate into one PSUM bank; the four
    gates share a [128, 4, bk] PSUM tensor ordered (i, f, o, g).
  * Layer 0 folds both time directions AND the biases into a single K=18
    augmented input ([x_t; 1; x_rt; 1]), so its activations need no bias:
    one sigmoid over the (i,f,o) span + one tanh for g.
  * Layers 1-2 keep per-gate activations with per-partition bias.
  * Elementwise rebalance: i*g and h=o*tanh(c) on DVE in bf16, f*c on the
    Pool engine, c accumulates in fp32 on DVE.
  * Final stage: bf16 PE transpose to natural layout + K=9 residual matmul
    into one PSUM bank, LayerNorm stats via one bn_stats + four bn_aggr on
    DVE, rstd via the approx reciprocal, normalization on Pool directly from
    PSUM, DMA out natural-layout fp32.
  * Hot-loop input DMAs issue from the Pool queue (cheap dispatch), output
    stores from the SP queue.
"""

from contextlib import ExitStack

import numpy as np
import ml_dtypes

import concourse.bacc as bacc
import concourse.tile as tile
from concourse import mybir
from concourse.bass_utils import run_bass_kernel_spmd

F32 = mybir.dt.float32
BF16 = mybir.dt.bfloat16
AF = mybir.ActivationFunctionType
OP = mybir.AluOpType

NCORES = 8
BC = 1024               # batch rows per core
CHUNKS = 2
T = 64
H = 64
NL = 3
D2 = 2 * H              # 128
LN_EPS = 1e-5

# gate order in PyTorch weights: i, f, g, o (rows g*H:(g+1)*H of w_ih/w_hh)
# on-chip slot order: g, i, f, o -- g first so its tanh clears the ACT queue
# early; f,o adjacent so layer-0 can sigmoid them as one span
PT2SLOT = {0: 1, 1: 2, 2: 0, 3: 3}   # pytorch gate idx -> PSUM slot
SLOT_G, SLOT_I, SLOT_F, SLOT_O = 0, 1, 2, 3


def _host_prep(x, w_ih, w_hh, b_ih, b_hh, w_res, b_res, ncores, bc):
    """Matmul-ready bf16 weight layouts (shared across cores) + per-core
    inputs."""
    x = np.asarray(x, np.float32)
    w_ih = np.asarray(w_ih, np.float32)
    w_hh = np.asarray(w_hh, np.float32)
    bias = np.asarray(b_ih, np.float32) + np.asarray(b_hh, np.float32)  # (NL,2,4H)
    w_res = np.asarray(w_res, np.float32)
    b_res = np.asarray(b_res, np.float32)
    t_len = x.shape[1]

    # Recurrent lhsT, K-major: rw[k, l, slot, m] (block-diagonal over dirs)
    rw = np.zeros((128, NL, 4, 128), np.float32)
    for l in range(NL):
        for g in range(4):
            s = PT2SLOT[g]
            gs = slice(g * H, (g + 1) * H)
            rw[0:64, l, s, 0:64] = w_hh[l, 0, gs, :].T
            rw[64:128, l, s, 64:128] = w_hh[l, 1, gs, :].T
    rw = rw.astype(ml_dtypes.bfloat16)

    # Input-projection lhsT for layers 1,2 (bf16): pw[k, l-1, slot, d, m]
    pw = np.zeros((128, NL - 1, 4, 2, 64), np.float32)
    for l in (1, 2):
        for g in range(4):
            s = PT2SLOT[g]
            gs = slice(g * H, (g + 1) * H)
            for d in range(2):
                pw[:, l - 1, s, d, :] = w_ih[l, d, gs, :].T
    pw = pw.astype(ml_dtypes.bfloat16)

    # Layer-0 fused fwd+bwd projection lhsT with biases on the ones-rows:
    # K rows 0:8 = x_t, row 8 = 1, rows 9:17 = x_rt, row 17 = 1.
    l0w = np.zeros((18, 4, 128), np.float32)
    for g in range(4):
        s = PT2SLOT[g]
        gs = slice(g * H, (g + 1) * H)
        l0w[0:8, s, 0:64] = w_ih[0, 0, gs, 0:8].T
        l0w[8, s, 0:64] = bias[0, 0, gs]
        l0w[9:17, s, 64:128] = w_ih[0, 1, gs, 0:8].T
        l0w[17, s, 64:128] = bias[0, 1, gs]
    l0w = l0w.astype(ml_dtypes.bfloat16)

    # per-partition gate biases, layers 1,2 (fused dirs): br[p, (l-1)*4+slot]
    br = np.zeros((128, (NL - 1) * 4), np.float32)
    for l in (1, 2):
        for g in range(4):
            s = PT2SLOT[g]
            gs = slice(g * H, (g + 1) * H)
            br[0:64, (l - 1) * 4 + s] = bias[l, 0, gs]
            br[64:128, (l - 1) * 4 + s] = bias[l, 1, gs]

    # residual rhs: wres[k, f] = w_res[f, k], row 8 = b_res
    wres = np.zeros((9, 128), np.float32)
    wres[0:8, :] = w_res.T
    wres[8, :] = b_res
    wres = wres.astype(ml_dtypes.bfloat16)

    ident = np.eye(128, dtype=np.float32)

    # Per-core transposed-augmented input: xaug[k, t, b] with both time
    # directions stacked: rows 0:8 = x_t, 8 = 1, 9:17 = x_{T-1-t}, 17 = 1
    xaug_cores = []
    for c in range(ncores):
        xc = x[c * bc:(c + 1) * bc]              # (bc, T, 8)
        xa = np.empty((18, t_len, bc), np.float32)
        xa[0:8] = xc.transpose(2, 1, 0)
        xa[8] = 1.0
        xa[9:17] = xc[:, ::-1].transpose(2, 1, 0)
        xa[17] = 1.0
        xaug_cores.append(xa.astype(ml_dtypes.bfloat16))

    shared = dict(rw=rw, pw=pw, l0w=l0w, br=br, wres=wres, ident=ident)
    return shared, xaug_cores


def _emit(nc, tc, ctx, D, apply_gb, bc, t_len):
    bk = bc // CHUNKS

    sbC = ctx.enter_context(tc.tile_pool(name="consts", bufs=1))
    sbA = ctx.enter_context(tc.tile_pool(name="workA", bufs=3))
    sbB = ctx.enter_context(tc.tile_pool(name="workB", bufs=2))
    sbS = ctx.enter_context(tc.tile_pool(name="state", bufs=1))
    ps = ctx.enter_context(tc.tile_pool(name="ps", bufs=1, space="PSUM"))

    def const_tile(shape, dtype, key):
        t = sbC.tile(shape, dtype, name=f"c_{key}", tag=f"c_{key}")
        nc.sync.dma_start(out=t, in_=D[key])
        return t

    rw_sb = const_tile([128, NL, 4, 128], BF16, "rw")
    pw_sb = const_tile([128, NL - 1, 4, 2, 64], BF16, "pw")
    l0w_sb = const_tile([18, 4, 128], BF16, "l0w")
    br_sb = const_tile([128, (NL - 1) * 4], F32, "br")
    wres_sb = const_tile([9, 128], BF16, "wres")
    ident_sb = const_tile([128, 128], F32, "ident")
    gamma_sb = beta_sb = None
    if apply_gb:
        gamma_sb = const_tile([128, 128], F32, "gammab")
        beta_sb = const_tile([128, 128], F32, "betab")
    eps_sb = sbC.tile([128, 1], F32)
    nc.vector.memset(eps_sb, LN_EPS)

    O = [D[f"o{i}"] for i in range(NL)]
    xaug = D["xaug"]
    out_d = D["out"]

    h_prev = [None] * CHUNKS
    c_st = [None] * CHUNKS

    def issue_inp(cc, l, k):
        c0 = cc * bk
        cols = slice(c0, c0 + bk)
        rt = t_len - 1 - k
        if l == 0:
            xin = sbA.tile([18, bk], BF16, tag=f"inf{cc}", bufs=4, name="xin")
            nc.gpsimd.dma_start(out=xin, in_=xaug[:, k, cols])
            return (xin,)
        inp_f = sbA.tile([128, bk], BF16, tag=f"inf{cc}", bufs=4, name="inp_f")
        nc.gpsimd.dma_start(out=inp_f, in_=O[l - 1][:, k, cols])
        inp_b = sbA.tile([128, bk], BF16, tag=f"inb{cc}", bufs=4, name="inp_b")
        nc.gpsimd.dma_start(out=inp_b, in_=O[l - 1][:, rt, cols])
        return (inp_f, inp_b)

    def lstm_step(cc, l, k, inps):
        G = ps.tile([128, 4, bk], F32, tag=f"g{cc}")

        for s in range(4):
            if l == 0:
                (xin,) = inps
                # fused fwd+bwd proj with bias rows, K=18
                nc.tensor.matmul(G[:, s, :], l0w_sb[:, s, :], xin,
                                 start=True, stop=(k == 0),
                                 skip_group_check=True)
            else:
                inp_f, inp_b = inps
                calls = [
                    (G[0:64, s, :], pw_sb[:, l - 1, s, 0, :], inp_f,
                     (0, 0), (0, 64)),
                    (G[64:128, s, :], pw_sb[:, l - 1, s, 1, :], inp_b,
                     (0, 64), (64, 128)),
                ]
                if k > 0:
                    calls.append((G[:, s, :], rw_sb[:, l, s, :], h_prev[cc],
                                  None, (0, 128)))
                n = len(calls)
                for i, (o, lh, rh, tp, rng) in enumerate(calls):
                    covered = set()
                    for _, _, _, _, r in calls[:i]:
                        covered.update(range(*r))
                    start = not set(range(*rng)).issubset(covered)
                    stop = not any(max(rng[0], r[0]) < min(rng[1], r[1])
                                   for _, _, _, _, r in calls[i + 1:])
                    # skip_group_check: the executing-sim group checker
                    # mis-addresses partition-based PSUM offsets (tensor rows
                    # != 16KB); data semantics are still simulated exactly.
                    nc.tensor.matmul(o, lh, rh, start=start, stop=stop,
                                     tile_position=tp, skip_group_check=True)
            if l == 0 and k > 0:
                nc.tensor.matmul(G[:, s, :], rw_sb[:, 0, s, :], h_prev[cc],
                                 start=False, stop=True,
                                 skip_group_check=True)

        S = sbB.tile([128, 4, bk], BF16, tag=f"s{cc}", bufs=3)
        if l == 0:
            # biases folded into the projection: bias-free activations; i
            # separate so i*g can start before the f,o sigmoids finish
            nc.scalar.activation(out=S[:, SLOT_G, :], in_=G[:, SLOT_G, :],
                                 func=AF.Tanh)
            nc.scalar.activation(out=S[:, SLOT_I, :], in_=G[:, SLOT_I, :],
                                 func=AF.Sigmoid)
            nc.scalar.activation(out=S[:, SLOT_F:SLOT_O + 1, :],
                                 in_=G[:, SLOT_F:SLOT_O + 1, :],
                                 func=AF.Sigmoid)
        else:
            b0 = (l - 1) * 4
            for s, fn in ((SLOT_G, AF.Tanh), (SLOT_I, AF.Sigmoid),
                          (SLOT_F, AF.Sigmoid), (SLOT_O, AF.Sigmoid)):
                nc.scalar.activation(out=S[:, s, :], in_=G[:, s, :], func=fn,
                                     bias=br_sb[:, b0 + s:b0 + s + 1])

        if k == 0:
            c = sbS.tile([128, bk], F32, tag=f"c{cc}")
            c_st[cc] = c
            nc.vector.tensor_mul(c, S[:, SLOT_I, :], S[:, SLOT_G, :])  # i*g
        else:
            c = c_st[cc]
            u = sbB.tile([128, bk], BF16, tag=f"u{cc}")
            nc.vector.tensor_mul(u, S[:, SLOT_I, :], S[:, SLOT_G, :])  # (DVE)
            nc.gpsimd.tensor_mul(c, S[:, SLOT_F, :], c)            # f*c (POOL)
            nc.gpsimd.tensor_add(c, c, u)                          # (POOL)
        return S, c

    def lstm_step_ph2(cc, l, k, S, c):
        # second phase emitted after the other chunk's phase 1 so the
        # ScalarE FIFO never head-of-line blocks on tanh(c) while the other
        # chunk's (ready) sigmoid sits behind it
        c0 = cc * bk
        cols = slice(c0, c0 + bk)
        rt = t_len - 1 - k
        Tc = sbB.tile([128, bk], BF16, tag=f"tc{cc}")
        nc.scalar.activation(out=Tc, in_=c, func=AF.Tanh)
        h = sbA.tile([128, bk], BF16, tag=f"h{cc}")
        nc.vector.tensor_mul(h, S[:, SLOT_O, :], Tc)               # h = o*tanh(c)
        h_prev[cc] = h

        # store time-ordered halves: fwd half at t=k, bwd half at t=rt
        nc.sync.dma_start(out=O[l][0:64, k, cols], in_=h[0:64, :])
        nc.sync.dma_start(out=O[l][64:128, rt, cols], in_=h[64:128, :])

    PF = min(2, t_len - 1)
    for l in range(NL):
        pend = {}
        for kk in range(PF):
            for cc in range(CHUNKS):
                pend[(cc, kk)] = issue_inp(cc, l, kk)
        for k in range(t_len):
            ph1 = {}
            for cc in range(CHUNKS):
                if k + PF < t_len:
                    pend[(cc, k + PF)] = issue_inp(cc, l, k + PF)
                ph1[cc] = lstm_step(cc, l, k, pend.pop((cc, k)))
            for cc in range(CHUNKS):
                S, c = ph1[cc]
                lstm_step_ph2(cc, l, k, S, c)

    # ---- final stage: relu + residual + LayerNorm + transpose to natural ----
    def issue_fin(cc, t):
        c0 = cc * bk
        cols = slice(c0, c0 + bk)
        o2t = sbA.tile([128, bk], BF16, tag=f"inf{cc}", bufs=4, name="o2t")
        nc.sync.dma_start(out=o2t, in_=O[NL - 1][:, t, cols])
        xt = sbA.tile([9, bk], BF16, tag=f"inb{cc}", bufs=4, name="xt")
        nc.sync.dma_start(out=xt, in_=xaug[0:9, t, cols])
        return o2t, xt

    def final_t(cc, t, o2t, xt):
        c0 = cc * bk
        relu4 = sbB.tile([128, bk], F32, tag=f"relu{cc}")
        nc.gpsimd.tensor_scalar_max(relu4, o2t, 0.0)
        # one accumulation group for the whole bank: transpose overwrites its
        # quarter (pending-zero from the single start), residual accumulates
        psZ = ps.tile([128, 4, 128], F32, tag=f"g{cc}")
        for bi in range(4):
            bs = slice(bi * 128, (bi + 1) * 128)
            nc.tensor.matmul(psZ[:, bi, :], relu4[:, bs], ident_sb,
                             is_transpose=True, start=(bi == 0), stop=False,
                             skip_group_check=True)
            nc.tensor.matmul(psZ[:, bi, :], xt[:, bs], wres_sb,
                             start=False, stop=(bi == 3),
                             skip_group_check=True)
        # z to SBUF once on the (otherwise idle) ScalarE, stats + normalize
        # read it from SBUF on DVE/Pool
        z = sbB.tile([128, 4, 128], F32, tag=f"z{cc}")
        nc.scalar.copy(out=z, in_=psZ)
        bnst = sbB.tile([128, 4, 6], F32, tag=f"bn{cc}")
        muvar = sbB.tile([128, 4, 2], F32, tag=f"mv{cc}")
        for bi in range(4):
            nc.vector.bn_stats(bnst[:, bi, :], z[:, bi, :])
            nc.vector.bn_aggr(muvar[:, bi, :], bnst[:, bi, :])
        sd = sbB.tile([128, 4, 1], F32, tag=f"sd{cc}")
        nc.scalar.activation(out=sd, in_=muvar[:, :, 1:2], func=AF.Sqrt,
                             bias=eps_sb)
        rstd = sbB.tile([128, 4, 1], F32, tag=f"rs{cc}")
        nc.vector.reciprocal_approx_fast(rstd, sd)
        nmr = sbB.tile([128, 4, 1], F32, tag=f"nm{cc}")
        nc.vector.scalar_tensor_tensor(nmr, muvar[:, :, 0:1], -1.0, rstd,
                                       op0=OP.mult, op1=OP.mult)
        onat = sbA.tile([128, 4, 128], F32, tag=f"on{cc}", bufs=3)
        for bi in range(4):
            nc.gpsimd.tensor_scalar(onat[:, bi, :], z[:, bi, :],
                                    rstd[:, bi, :], nmr[:, bi, :],
                                    op0=OP.mult, op1=OP.add)
            if apply_gb:
                nc.gpsimd.tensor_mul(onat[:, bi, :], onat[:, bi, :], gamma_sb)
                nc.gpsimd.tensor_add(onat[:, bi, :], onat[:, bi, :], beta_sb)
        oap = out_d[c0:c0 + bk, t, :].rearrange("(b p) f -> p b f", p=128)
        nc.sync.dma_start(out=oap, in_=onat)

    fpend = {}
    for tt in range(PF):
        for cc in range(CHUNKS):
            fpend[(cc, tt)] = issue_fin(cc, tt)
    for t in range(t_len):
        for cc in range(CHUNKS):
            if t + PF < t_len:
                fpend[(cc, t + PF)] = issue_fin(cc, t + PF)
            o2t, xt = fpend.pop((cc, t))
            final_t(cc, t, o2t, xt)


def build(apply_gb=False, bc=BC, t_len=T, num_devices=NCORES):
    nc = bacc.Bacc("TRN2", target_bir_lowering=False, debug=False,
                   num_devices=num_devices)
    D = {}

    def inp(name, shape, dtype=F32):
        D[name] = nc.dram_tensor(name, shape, dtype, kind="ExternalInput").ap()

    inp("xaug", [18, t_len, bc], BF16)
    inp("rw", [128, NL, 4, 128], BF16)
    inp("pw", [128, NL - 1, 4, 2, 64], BF16)
    inp("l0w", [18, 4, 128], BF16)
    inp("br", [128, (NL - 1) * 4])
    inp("wres", [9, 128], BF16)
    inp("ident", [128, 128])
    if apply_gb:
        inp("gammab", [128, 128])
        inp("betab", [128, 128])
    for i in range(NL):
        D[f"o{i}"] = nc.dram_tensor(f"o{i}", [128, t_len, bc], BF16).ap()
    D["out"] = nc.dram_tensor("out", [bc, t_len, 128], F32,
                              kind="ExternalOutput").ap()

    with tile.TileContext(nc) as tc:
        with ExitStack() as ctx:
            _emit(nc, tc, ctx, D, apply_gb, bc, t_len)
    nc.compile()
    return nc


_BUILD_CACHE = {}


def make_in_maps(inputs, apply_gb):
    ln_gamma = np.asarray(inputs["ln_gamma"], np.float32)
    ln_beta = np.asarray(inputs["ln_beta"], np.float32)
    shared, xaug_cores = _host_prep(
        inputs["x"], inputs["w_ih"], inputs["w_hh"], inputs["b_ih"],
        inputs["b_hh"], inputs["w_res"], inputs["b_res"], NCORES, BC)
    in_maps = []
    for c in range(NCORES):
        m = dict(shared)
        m["xaug"] = xaug_cores[c]
        if apply_gb:
            m["gammab"] = np.ascontiguousarray(
                np.broadcast_to(ln_gamma, (128, 128)).astype(np.float32))
            m["betab"] = np.ascontiguousarray(
                np.broadcast_to(ln_beta, (128, 128)).astype(np.float32))
        in_maps.append(m)
    return in_maps


def kernel(x, w_ih, w_hh, b_ih, b_hh, w_res, b_res, ln_gamma, ln_beta):
    ln_gamma = np.asarray(ln_gamma, np.float32)
    ln_beta = np.asarray(ln_beta, np.float32)
    apply_gb = not (np.all(ln_gamma == 1.0) and np.all(ln_beta == 0.0))

    if apply_gb not in _BUILD_CACHE:
        _BUILD_CACHE[apply_gb] = build(apply_gb)
    nc = _BUILD_CACHE[apply_gb]

    inputs = dict(x=x, w_ih=w_ih, w_hh=w_hh, b_ih=b_ih, b_hh=b_hh,
                  w_res=w_res, b_res=b_res, ln_gamma=ln_gamma, ln_beta=ln_beta)
    in_maps = make_in_maps(inputs, apply_gb)

    res = run_bass_kernel_spmd(nc, in_maps, core_ids=list(range(NCORES)))
    out = np.concatenate([res.results[c]["out"] for c in range(NCORES)], axis=0)
    return np.ascontiguousarray(out.astype(np.float32))


# revision 15
# speedup vs baseline: 1.9650x; 1.0083x over previous
"""Trainium2 Bass kernel for a 3-layer BiLSTM + ReLU + residual + LayerNorm.

Strategy (pure data parallel over 8 cores, 1024 batch rows per core):
  * "Transposed" layout on-chip: features on SBUF partitions, batch on the
    free dim.  Both directions fused on partitions (fwd = 0:64, bwd = 64:128)
    so every engine op runs with all 128 lanes busy.
  * All matmuls in bf16 (4x cheaper per row than fp32 on the PE): per
    timestep, per gate, a col-tiled input-projection pair plus one 128x128
    block-diagonal recurrent matmul accumulate into one PSUM bank; the four
    gates share a [128, 4, bk] PSUM tensor ordered (i, f, o, g).
  * Layer 0 folds both time directions AND the biases into a single K=18
    augmented input ([x_t; 1; x_rt; 1]), so its activations need no bias:
    one sigmoid over the (i,f,o) span + one tanh for g.
  * Layers 1-2 keep per-gate activations with per-partition bias.
  * Elementwise rebalance: i*g and h=o*tanh(c) on DVE in bf16, f*c on the
    Pool engine, c accumulates in fp32 on DVE.
  * Final stage: bf16 PE transpose to natural layout + K=9 residual matmul
    into one PSUM bank, LayerNorm stats via one bn_stats + four bn_aggr on
    DVE, rstd via the approx reciprocal, normalization on Pool directly from
    PSUM, DMA out natural-layout fp32.
  * Hot-loop input DMAs issue from the Pool queue (cheap dispatch), output
    stores from the SP queue.
"""

from contextlib import ExitStack

import numpy as np
import ml_dtypes

import concourse.bacc as bacc
import concourse.tile as tile
from concourse import mybir
from concourse.bass_utils import run_bass_kernel_spmd

F32 = mybir.dt.float32
BF16 = mybir.dt.bfloat16
AF = mybir.ActivationFunctionType
OP = mybir.AluOpType

NCORES = 8
BC = 1024               # batch rows per core
CHUNKS = 2
T = 64
H = 64
NL = 3
D2 = 2 * H              # 128
LN_EPS = 1e-5

# gate order in PyTorch weights: i, f, g, o (rows g*H:(g+1)*H of w_ih/w_hh)
# on-chip slot order: g, i, f, o -- g first so its tanh clears the ACT queue
# early; f,o adjacent so layer-0 can sigmoid them as one span
PT2SLOT = {0: 1, 1: 2, 2: 0, 3: 3}   # pytorch gate idx -> PSUM slot
SLOT_G, SLOT_I, SLOT_F, SLOT_O = 0, 1, 2, 3


def _host_prep(x, w_ih, w_hh, b_ih, b_hh, w_res, b_res, ncores, bc):
    """Matmul-ready bf16 weight layouts (shared across cores) + per-core
    inputs."""
    x = np.asarray(x, np.float32)
    w_ih = np.asarray(w_ih, np.float32)
    w_hh = np.asarray(w_hh, np.float32)
    bias = np.asarray(b_ih, np.float32) + np.asarray(b_hh, np.float32)  # (NL,2,4H)
    w_res = np.asarray(w_res, np.float32)
    b_res = np.asarray(b_res, np.float32)
    t_len = x.shape[1]

    # Recurrent lhsT, K-major: rw[k, l, slot, m] (block-diagonal over dirs)
    rw = np.zeros((128, NL, 4, 128), np.float32)
    for l in range(NL):
        for g in range(4):
            s = PT2SLOT[g]
            gs = slice(g * H, (g + 1) * H)
            rw[0:64, l, s, 0:64] = w_hh[l, 0, gs, :].T
            rw[64:128, l, s, 64:128] = w_hh[l, 1, gs, :].T
    rw = rw.astype(ml_dtypes.bfloat16)

    # Input-projection lhsT for layers 1,2 (bf16): pw[k, l-1, slot, d, m]
    pw = np.zeros((128, NL - 1, 4, 2, 64), np.float32)
    for l in (1, 2):
        for g in range(4):
            s = PT2SLOT[g]
            gs = slice(g * H, (g + 1) * H)
            for d in range(2):
                pw[:, l - 1, s, d, :] = w_ih[l, d, gs, :].T
    pw = pw.astype(ml_dtypes.bfloat16)

    # Layer-0 fused fwd+bwd projection lhsT with biases on the ones-rows:
    # K rows 0:8 = x_t, row 8 = 1, rows 9:17 = x_rt, row 17 = 1.
    l0w = np.zeros((18, 4, 128), np.float32)
    for g in range(4):
        s = PT2SLOT[g]
        gs = slice(g * H, (g + 1) * H)
        l0w[0:8, s, 0:64] = w_ih[0, 0, gs, 0:8].T
        l0w[8, s, 0:64] = bias[0, 0, gs]
        l0w[9:17, s, 64:128] = w_ih[0, 1, gs, 0:8].T
        l0w[17, s, 64:128] = bias[0, 1, gs]
    l0w = l0w.astype(ml_dtypes.bfloat16)

    # per-partition gate biases, layers 1,2 (fused dirs): br[p, (l-1)*4+slot]
    br = np.zeros((128, (NL - 1) * 4), np.float32)
    for l in (1, 2):
        for g in range(4):
            s = PT2SLOT[g]
            gs = slice(g * H, (g + 1) * H)
            br[0:64, (l - 1) * 4 + s] = bias[l, 0, gs]
            br[64:128, (l - 1) * 4 + s] = bias[l, 1, gs]

    # residual rhs: wres[k, f] = w_res[f, k], row 8 = b_res
    wres = np.zeros((9, 128), np.float32)
    wres[0:8, :] = w_res.T
    wres[8, :] = b_res
    wres = wres.astype(ml_dtypes.bfloat16)

    ident = np.eye(128, dtype=np.float32)

    # Per-core transposed-augmented input: xaug[k, t, b] with both time
    # directions stacked: rows 0:8 = x_t, 8 = 1, 9:17 = x_{T-1-t}, 17 = 1
    xaug_cores = []
    for c in range(ncores):
        xc = x[c * bc:(c + 1) * bc]              # (bc, T, 8)
        xa = np.empty((18, t_len, bc), np.float32)
        xa[0:8] = xc.transpose(2, 1, 0)
        xa[8] = 1.0
        xa[9:17] = xc[:, ::-1].transpose(2, 1, 0)
        xa[17] = 1.0
        xaug_cores.append(xa.astype(ml_dtypes.bfloat16))

    shared = dict(rw=rw, pw=pw, l0w=l0w, br=br, wres=wres, ident=ident)
    return shared, xaug_cores


def _emit(nc, tc, ctx, D, apply_gb, bc, t_len):
    bk = bc // CHUNKS

    sbC = ctx.enter_context(tc.tile_pool(name="consts", bufs=1))
    sbA = ctx.enter_context(tc.tile_pool(name="workA", bufs=3))
    sbB = ctx.enter_context(tc.tile_pool(name="workB", bufs=2))
    sbS = ctx.enter_context(tc.tile_pool(name="state", bufs=1))
    ps = ctx.enter_context(tc.tile_pool(name="ps", bufs=1, space="PSUM"))

    def const_tile(shape, dtype, key):
        t = sbC.tile(shape, dtype, name=f"c_{key}", tag=f"c_{key}")
        nc.sync.dma_start(out=t, in_=D[key])
        return t

    rw_sb = const_tile([128, NL, 4, 128], BF16, "rw")
    pw_sb = const_tile([128, NL - 1, 4, 2, 64], BF16, "pw")
    l0w_sb = const_tile([18, 4, 128], BF16, "l0w")
    br_sb = const_tile([128, (NL - 1) * 4], F32, "br")
    wres_sb = const_tile([9, 128], BF16, "wres")
    ident_sb = const_tile([128, 128], F32, "ident")
    gamma_sb = beta_sb = None
    if apply_gb:
        gamma_sb = const_tile([128, 128], F32, "gammab")
        beta_sb = const_tile([128, 128], F32, "betab")
    eps_sb = sbC.tile([128, 1], F32)
    nc.vector.memset(eps_sb, LN_EPS)

    O = [D[f"o{i}"] for i in range(NL)]
    xaug = D["xaug"]
    out_d = D["out"]

    h_prev = [None] * CHUNKS
    c_st = [None] * CHUNKS

    def issue_inp(cc, l, k):
        c0 = cc * bk
        cols = slice(c0, c0 + bk)
        rt = t_len - 1 - k
        if l == 0:
            xin = sbA.tile([18, bk], BF16, tag=f"inf{cc}", bufs=4, name="xin")
            nc.gpsimd.dma_start(out=xin, in_=xaug[:, k, cols])
            return (xin,)
        # both time slices (k and rt) in one strided DMA; half order follows
        # the slice direction
        t0, t1 = (k, rt) if k < rt else (rt, k)
        inp2 = sbA.tile([128, 2, bk], BF16, tag=f"inf{cc}", bufs=4,
                        name="inp2")
        nc.gpsimd.dma_start(out=inp2,
                            in_=O[l - 1][:, t0:t1 + 1:t1 - t0, cols])
        if k < rt:
            return (inp2[:, 0, :], inp2[:, 1, :])
        return (inp2[:, 1, :], inp2[:, 0, :])

    def lstm_step(cc, l, k, inps):
        G = ps.tile([128, 4, bk], F32, tag=f"g{cc}")

        for s in range(4):
            if l == 0:
                (xin,) = inps
                # fused fwd+bwd proj with bias rows, K=18
                nc.tensor.matmul(G[:, s, :], l0w_sb[:, s, :], xin,
                                 start=True, stop=(k == 0),
                                 skip_group_check=True)
            else:
                inp_f, inp_b = inps
                calls = [
                    (G[0:64, s, :], pw_sb[:, l - 1, s, 0, :], inp_f,
                     (0, 0), (0, 64)),
                    (G[64:128, s, :], pw_sb[:, l - 1, s, 1, :], inp_b,
                     (0, 64), (64, 128)),
                ]
                if k > 0:
                    calls.append((G[:, s, :], rw_sb[:, l, s, :], h_prev[cc],
                                  None, (0, 128)))
                n = len(calls)
                for i, (o, lh, rh, tp, rng) in enumerate(calls):
                    covered = set()
                    for _, _, _, _, r in calls[:i]:
                        covered.update(range(*r))
                    start = not set(range(*rng)).issubset(covered)
                    stop = not any(max(rng[0], r[0]) < min(rng[1], r[1])
                                   for _, _, _, _, r in calls[i + 1:])
                    # skip_group_check: the executing-sim group checker
                    # mis-addresses partition-based PSUM offsets (tensor rows
                    # != 16KB); data semantics are still simulated exactly.
                    nc.tensor.matmul(o, lh, rh, start=start, stop=stop,
                                     tile_position=tp, skip_group_check=True)
            if l == 0 and k > 0:
                nc.tensor.matmul(G[:, s, :], rw_sb[:, 0, s, :], h_prev[cc],
                                 start=False, stop=True,
                                 skip_group_check=True)

        S = sbB.tile([128, 4, bk], BF16, tag=f"s{cc}", bufs=3)
        if l == 0:
            # biases folded into the projection: bias-free activations; i
            # separate so i*g can start before the f,o sigmoids finish
            nc.scalar.activation(out=S[:, SLOT_G, :], in_=G[:, SLOT_G, :],
                                 func=AF.Tanh)
            nc.scalar.activation(out=S[:, SLOT_I, :], in_=G[:, SLOT_I, :],
                                 func=AF.Sigmoid)
            nc.scalar.activation(out=S[:, SLOT_F:SLOT_O + 1, :],
                                 in_=G[:, SLOT_F:SLOT_O + 1, :],
                                 func=AF.Sigmoid)
        else:
            b0 = (l - 1) * 4
            for s, fn in ((SLOT_G, AF.Tanh), (SLOT_I, AF.Sigmoid),
                          (SLOT_F, AF.Sigmoid), (SLOT_O, AF.Sigmoid)):
                nc.scalar.activation(out=S[:, s, :], in_=G[:, s, :], func=fn,
                                     bias=br_sb[:, b0 + s:b0 + s + 1])

        if k == 0:
            c = sbS.tile([128, bk], F32, tag=f"c{cc}")
            c_st[cc] = c
            nc.vector.tensor_mul(c, S[:, SLOT_I, :], S[:, SLOT_G, :])  # i*g
        else:
            c = c_st[cc]
            u = sbB.tile([128, bk], BF16, tag=f"u{cc}")
            nc.vector.tensor_mul(u, S[:, SLOT_I, :], S[:, SLOT_G, :])  # (DVE)
            nc.gpsimd.tensor_mul(c, S[:, SLOT_F, :], c)            # f*c (POOL)
            nc.gpsimd.tensor_add(c, c, u)                          # (POOL)
        return S, c

    def lstm_step_ph2(cc, l, k, S, c):
        # second phase emitted after the other chunk's phase 1 so the
        # ScalarE FIFO never head-of-line blocks on tanh(c) while the other
        # chunk's (ready) sigmoid sits behind it
        c0 = cc * bk
        cols = slice(c0, c0 + bk)
        rt = t_len - 1 - k
        Tc = sbB.tile([128, bk], BF16, tag=f"tc{cc}")
        nc.scalar.activation(out=Tc, in_=c, func=AF.Tanh)
        h = sbA.tile([128, bk], BF16, tag=f"h{cc}")
        nc.vector.tensor_mul(h, S[:, SLOT_O, :], Tc)               # h = o*tanh(c)
        h_prev[cc] = h

        # store time-ordered halves: fwd half at t=k, bwd half at t=rt
        nc.sync.dma_start(out=O[l][0:64, k, cols], in_=h[0:64, :])
        nc.sync.dma_start(out=O[l][64:128, rt, cols], in_=h[64:128, :])

    PF = min(2, t_len - 1)
    for l in range(NL):
        pend = {}
        for kk in range(PF):
            for cc in range(CHUNKS):
                pend[(cc, kk)] = issue_inp(cc, l, kk)
        for k in range(t_len):
            ph1 = {}
            for cc in range(CHUNKS):
                if k + PF < t_len:
                    pend[(cc, k + PF)] = issue_inp(cc, l, k + PF)
                ph1[cc] = lstm_step(cc, l, k, pend.pop((cc, k)))
            for cc in range(CHUNKS):
                S, c = ph1[cc]
                lstm_step_ph2(cc, l, k, S, c)

    # ---- final stage: relu + residual + LayerNorm + transpose to natural ----
    def issue_fin(cc, t):
        c0 = cc * bk
        cols = slice(c0, c0 + bk)
        o2t = sbA.tile([128, bk], BF16, tag=f"inf{cc}", bufs=4, name="o2t")
        nc.gpsimd.dma_start(out=o2t, in_=O[NL - 1][:, t, cols])
        xt = sbA.tile([9, bk], BF16, tag=f"inb{cc}", bufs=4, name="xt")
        nc.scalar.dma_start(out=xt, in_=xaug[0:9, t, cols])
        return o2t, xt

    def final_t(cc, t, o2t, xt):
        c0 = cc * bk
        relu4 = sbB.tile([128, bk], F32, tag=f"relu{cc}")
        nc.vector.tensor_scalar_max(relu4, o2t, 0.0)
        # one accumulation group for the whole bank: transpose overwrites its
        # quarter (pending-zero from the single start), residual accumulates
        psZ = ps.tile([128, 4, 128], F32, tag=f"g{cc}")
        for bi in range(4):
            bs = slice(bi * 128, (bi + 1) * 128)
            nc.tensor.matmul(psZ[:, bi, :], relu4[:, bs], ident_sb,
                             is_transpose=True, start=(bi == 0), stop=False,
                             skip_group_check=True)
            nc.tensor.matmul(psZ[:, bi, :], xt[:, bs], wres_sb,
                             start=False, stop=(bi == 3),
                             skip_group_check=True)
        # z to SBUF once on the (otherwise idle) ScalarE, stats + normalize
        # read it from SBUF on DVE/Pool
        z = sbB.tile([128, 4, 128], F32, tag=f"z{cc}")
        nc.scalar.copy(out=z, in_=psZ)
        bnst = sbB.tile([128, 4, 6], F32, tag=f"bn{cc}")
        muvar = sbB.tile([128, 4, 2], F32, tag=f"mv{cc}")
        for bi in range(4):
            nc.vector.bn_stats(bnst[:, bi, :], z[:, bi, :])
            nc.vector.bn_aggr(muvar[:, bi, :], bnst[:, bi, :])
        sd = sbB.tile([128, 4, 1], F32, tag=f"sd{cc}")
        nc.scalar.activation(out=sd, in_=muvar[:, :, 1:2], func=AF.Sqrt,
                             bias=eps_sb)
        rstd = sbB.tile([128, 4, 1], F32, tag=f"rs{cc}")
        nc.vector.reciprocal_approx_fast(rstd, sd)
        nmr = sbB.tile([128, 4, 1], F32, tag=f"nm{cc}")
        nc.vector.scalar_tensor_tensor(nmr, muvar[:, :, 0:1], -1.0, rstd,
                                       op0=OP.mult, op1=OP.mult)
        onat = sbA.tile([128, 4, 128], F32, tag=f"on{cc}", bufs=3)
        for bi in range(4):
            nc.gpsimd.tensor_scalar(onat[:, bi, :], z[:, bi, :],
                                    rstd[:, bi, :], nmr[:, bi, :],
                                    op0=OP.mult, op1=OP.add)
            if apply_gb:
                nc.gpsimd.tensor_mul(onat[:, bi, :], onat[:, bi, :], gamma_sb)
                nc.gpsimd.tensor_add(onat[:, bi, :], onat[:, bi, :], beta_sb)
        oap = out_d[c0:c0 + bk, t, :].rearrange("(b p) f -> p b f", p=128)
        nc.sync.dma_start(out=oap, in_=onat)

    fpend = {}
    for tt in range(PF):
        for cc in range(CHUNKS):
            fpend[(cc, tt)] = issue_fin(cc, tt)
    for t in range(t_len):
        for cc in range(CHUNKS):
            if t + PF < t_len:
                fpend[(cc, t + PF)] = issue_fin(cc, t + PF)
            o2t, xt = fpend.pop((cc, t))
            final_t(cc, t, o2t, xt)


def build(apply_gb=False, bc=BC, t_len=T, num_devices=NCORES):
    nc = bacc.Bacc("TRN2", target_bir_lowering=False, debug=False,
                   num_devices=num_devices)
    D = {}

    def inp(name, shape, dtype=F32):
        D[name] = nc.dram_tensor(name, shape, dtype, kind="ExternalInput").ap()

    inp("xaug", [18, t_len, bc], BF16)
    inp("rw", [128, NL, 4, 128], BF16)
    inp("pw", [128, NL - 1, 4, 2, 64], BF16)
    inp("l0w", [18, 4, 128], BF16)
    inp("br", [128, (NL - 1) * 4])
    inp("wres", [9, 128], BF16)
    inp("ident", [128, 128])
    if apply_gb:
        inp("gammab", [128, 128])
        inp("betab", [128, 128])
    for i in range(NL):
        D[f"o{i}"] = nc.dram_tensor(f"o{i}", [128, t_len, bc], BF16).ap()
    D["out"] = nc.dram_tensor("out", [bc, t_len, 128], F32,
                              kind="ExternalOutput").ap()

    with tile.TileContext(nc) as tc:
        with ExitStack() as ctx:
            _emit(nc, tc, ctx, D, apply_gb, bc, t_len)
    nc.compile()
    return nc


_BUILD_CACHE = {}


def make_in_maps(inputs, apply_gb):
    ln_gamma = np.asarray(inputs["ln_gamma"], np.float32)
    ln_beta = np.asarray(inputs["ln_beta"], np.float32)
    shared, xaug_cores = _host_prep(
        inputs["x"], inputs["w_ih"], inputs["w_hh"], inputs["b_ih"],
        inputs["b_hh"], inputs["w_res"], inputs["b_res"], NCORES, BC)
    in_maps = []
    for c in range(NCORES):
        m = dict(shared)
        m["xaug"] = xaug_cores[c]
        if apply_gb:
            m["gammab"] = np.ascontiguousarray(
                np.broadcast_to(ln_gamma, (128, 128)).astype(np.float32))
            m["betab"] = np.ascontiguousarray(
                np.broadcast_to(ln_beta, (128, 128)).astype(np.float32))
        in_maps.append(m)
    return in_maps


def kernel(x, w_ih, w_hh, b_ih, b_hh, w_res, b_res, ln_gamma, ln_beta):
    ln_gamma = np.asarray(ln_gamma, np.float32)
    ln_beta = np.asarray(ln_beta, np.float32)
    apply_gb = not (np.all(ln_gamma == 1.0) and np.all(ln_beta == 0.0))

    if apply_gb not in _BUILD_CACHE:
        _BUILD_CACHE[apply_gb] = build(apply_gb)
    nc = _BUILD_CACHE[apply_gb]

    inputs = dict(x=x, w_ih=w_ih, w_hh=w_hh, b_ih=b_ih, b_hh=b_hh,
                  w_res=w_res, b_res=b_res, ln_gamma=ln_gamma, ln_beta=ln_beta)
    in_maps = make_in_maps(inputs, apply_gb)

    res = run_bass_kernel_spmd(nc, in_maps, core_ids=list(range(NCORES)))
    out = np.concatenate([res.results[c]["out"] for c in range(NCORES)], axis=0)
    return np.ascontiguousarray(out.astype(np.float32))


# revision 17
# speedup vs baseline: 1.9875x; 1.0115x over previous
"""Trainium2 Bass kernel for a 3-layer BiLSTM + ReLU + residual + LayerNorm.

Strategy (pure data parallel over 8 cores, 1024 batch rows per core):
  * "Transposed" layout on-chip: features on SBUF partitions, batch on the
    free dim.  Both directions fused on partitions (fwd = 0:64, bwd = 64:128)
    so every engine op runs with all 128 lanes busy.
  * All matmuls in bf16 (4x cheaper per row than fp32 on the PE): per
    timestep, per gate, a col-tiled input-projection pair plus one 128x128
    block-diagonal recurrent matmul accumulate into one PSUM bank; the four
    gates share a [128, 4, bk] PSUM tensor ordered (i, f, o, g).
  * Layer 0 folds both time directions AND the biases into a single K=18
    augmented input ([x_t; 1; x_rt; 1]), so its activations need no bias:
    one sigmoid over the (i,f,o) span + one tanh for g.
  * Layers 1-2 keep per-gate activations with per-partition bias.
  * Elementwise rebalance: i*g and h=o*tanh(c) on DVE in bf16, f*c on the
    Pool engine, c accumulates in fp32 on DVE.
  * Final stage: bf16 PE transpose to natural layout + K=9 residual matmul
    into one PSUM bank, LayerNorm stats via one bn_stats + four bn_aggr on
    DVE, rstd via the approx reciprocal, normalization on Pool directly from
    PSUM, DMA out natural-layout fp32.
  * Hot-loop input DMAs issue from the Pool queue (cheap dispatch), output
    stores from the SP queue.
"""

from contextlib import ExitStack

import numpy as np
import ml_dtypes

import concourse.bacc as bacc
import concourse.tile as tile
from concourse import mybir
from concourse.bass_utils import run_bass_kernel_spmd

F32 = mybir.dt.float32
BF16 = mybir.dt.bfloat16
AF = mybir.ActivationFunctionType
OP = mybir.AluOpType

NCORES = 8
BC = 1024               # batch rows per core
CHUNKS = 2
T = 64
H = 64
NL = 3
D2 = 2 * H              # 128
LN_EPS = 1e-5

# gate order in PyTorch weights: i, f, g, o (rows g*H:(g+1)*H of w_ih/w_hh)
# on-chip slot order: g, f, i, o -- g first so its tanh clears the ACT queue
# early, f second so the Pool f*c starts before the i/o sigmoids; i,o
# adjacent so layer-0 can sigmoid them as one span
PT2SLOT = {0: 2, 1: 1, 2: 0, 3: 3}   # pytorch gate idx -> PSUM slot
SLOT_G, SLOT_F, SLOT_I, SLOT_O = 0, 1, 2, 3


def _host_prep(x, w_ih, w_hh, b_ih, b_hh, w_res, b_res, ncores, bc):
    """Matmul-ready bf16 weight layouts (shared across cores) + per-core
    inputs."""
    x = np.asarray(x, np.float32)
    w_ih = np.asarray(w_ih, np.float32)
    w_hh = np.asarray(w_hh, np.float32)
    bias = np.asarray(b_ih, np.float32) + np.asarray(b_hh, np.float32)  # (NL,2,4H)
    w_res = np.asarray(w_res, np.float32)
    b_res = np.asarray(b_res, np.float32)
    t_len = x.shape[1]

    # Recurrent lhsT, K-major: rw[k, l, slot, m] (block-diagonal over dirs)
    rw = np.zeros((128, NL, 4, 128), np.float32)
    for l in range(NL):
        for g in range(4):
            s = PT2SLOT[g]
            gs = slice(g * H, (g + 1) * H)
            rw[0:64, l, s, 0:64] = w_hh[l, 0, gs, :].T
            rw[64:128, l, s, 64:128] = w_hh[l, 1, gs, :].T
    rw = rw.astype(ml_dtypes.bfloat16)

    # Input-projection lhsT for layers 1,2 (bf16): pw[k, l-1, slot, d, m]
    pw = np.zeros((128, NL - 1, 4, 2, 64), np.float32)
    for l in (1, 2):
        for g in range(4):
            s = PT2SLOT[g]
            gs = slice(g * H, (g + 1) * H)
            for d in range(2):
                pw[:, l - 1, s, d, :] = w_ih[l, d, gs, :].T
    pw = pw.astype(ml_dtypes.bfloat16)

    # Layer-0 fused fwd+bwd projection lhsT with biases on the ones-rows:
    # K rows 0:8 = x_t, row 8 = 1, rows 9:17 = x_rt, row 17 = 1.
    l0w = np.zeros((18, 4, 128), np.float32)
    for g in range(4):
        s = PT2SLOT[g]
        gs = slice(g * H, (g + 1) * H)
        l0w[0:8, s, 0:64] = w_ih[0, 0, gs, 0:8].T
        l0w[8, s, 0:64] = bias[0, 0, gs]
        l0w[9:17, s, 64:128] = w_ih[0, 1, gs, 0:8].T
        l0w[17, s, 64:128] = bias[0, 1, gs]
    l0w = l0w.astype(ml_dtypes.bfloat16)

    # per-partition gate biases, layers 1,2 (fused dirs): br[p, (l-1)*4+slot]
    br = np.zeros((128, (NL - 1) * 4), np.float32)
    for l in (1, 2):
        for g in range(4):
            s = PT2SLOT[g]
            gs = slice(g * H, (g + 1) * H)
            br[0:64, (l - 1) * 4 + s] = bias[l, 0, gs]
            br[64:128, (l - 1) * 4 + s] = bias[l, 1, gs]

    # residual rhs: wres[k, f] = w_res[f, k], row 8 = b_res
    wres = np.zeros((9, 128), np.float32)
    wres[0:8, :] = w_res.T
    wres[8, :] = b_res
    wres = wres.astype(ml_dtypes.bfloat16)

    ident = np.eye(128, dtype=np.float32)

    # Per-core transposed-augmented input: xaug[k, t, b] with both time
    # directions stacked: rows 0:8 = x_t, 8 = 1, 9:17 = x_{T-1-t}, 17 = 1
    xaug_cores = []
    for c in range(ncores):
        xc = x[c * bc:(c + 1) * bc]              # (bc, T, 8)
        xa = np.empty((18, t_len, bc), np.float32)
        xa[0:8] = xc.transpose(2, 1, 0)
        xa[8] = 1.0
        xa[9:17] = xc[:, ::-1].transpose(2, 1, 0)
        xa[17] = 1.0
        xaug_cores.append(xa.astype(ml_dtypes.bfloat16))

    shared = dict(rw=rw, pw=pw, l0w=l0w, br=br, wres=wres, ident=ident)
    return shared, xaug_cores


def _emit(nc, tc, ctx, D, apply_gb, bc, t_len):
    bk = bc // CHUNKS

    sbC = ctx.enter_context(tc.tile_pool(name="consts", bufs=1))
    sbA = ctx.enter_context(tc.tile_pool(name="workA", bufs=3))
    sbB = ctx.enter_context(tc.tile_pool(name="workB", bufs=2))
    sbS = ctx.enter_context(tc.tile_pool(name="state", bufs=1))
    ps = ctx.enter_context(tc.tile_pool(name="ps", bufs=1, space="PSUM"))

    def const_tile(shape, dtype, key):
        t = sbC.tile(shape, dtype, name=f"c_{key}", tag=f"c_{key}")
        nc.sync.dma_start(out=t, in_=D[key])
        return t

    rw_sb = const_tile([128, NL, 4, 128], BF16, "rw")
    pw_sb = const_tile([128, NL - 1, 4, 2, 64], BF16, "pw")
    l0w_sb = const_tile([18, 4, 128], BF16, "l0w")
    br_sb = const_tile([128, (NL - 1) * 4], F32, "br")
    wres_sb = const_tile([9, 128], BF16, "wres")
    ident_sb = const_tile([128, 128], F32, "ident")
    gamma_sb = beta_sb = None
    if apply_gb:
        gamma_sb = const_tile([128, 128], F32, "gammab")
        beta_sb = const_tile([128, 128], F32, "betab")
    eps_sb = sbC.tile([128, 1], F32)
    nc.vector.memset(eps_sb, LN_EPS)

    O = [D[f"o{i}"] for i in range(NL)]
    xaug = D["xaug"]
    out_d = D["out"]

    h_prev = [None] * CHUNKS
    c_st = [None] * CHUNKS

    def issue_inp(cc, l, k):
        c0 = cc * bk
        cols = slice(c0, c0 + bk)
        rt = t_len - 1 - k
        if l == 0:
            xin = sbA.tile([18, bk], BF16, tag=f"inf{cc}", bufs=4, name="xin")
            nc.gpsimd.dma_start(out=xin, in_=xaug[:, k, cols])
            return (xin,)
        # both time slices (k and rt) in one strided DMA; half order follows
        # the slice direction
        t0, t1 = (k, rt) if k < rt else (rt, k)
        inp2 = sbA.tile([128, 2, bk], BF16, tag=f"inf{cc}", bufs=4,
                        name="inp2")
        nc.gpsimd.dma_start(out=inp2,
                            in_=O[l - 1][:, t0:t1 + 1:t1 - t0, cols])
        if k < rt:
            return (inp2[:, 0, :], inp2[:, 1, :])
        return (inp2[:, 1, :], inp2[:, 0, :])

    def lstm_step(cc, l, k, inps):
        G = ps.tile([128, 4, bk], F32, tag=f"g{cc}")

        for s in range(4):
            if l == 0:
                (xin,) = inps
                # fused fwd+bwd proj with bias rows, K=18
                nc.tensor.matmul(G[:, s, :], l0w_sb[:, s, :], xin,
                                 start=True, stop=(k == 0),
                                 skip_group_check=True)
            else:
                inp_f, inp_b = inps
                calls = [
                    (G[0:64, s, :], pw_sb[:, l - 1, s, 0, :], inp_f,
                     (0, 0), (0, 64)),
                    (G[64:128, s, :], pw_sb[:, l - 1, s, 1, :], inp_b,
                     (0, 64), (64, 128)),
                ]
                if k > 0:
                    calls.append((G[:, s, :], rw_sb[:, l, s, :], h_prev[cc],
                                  None, (0, 128)))
                n = len(calls)
                for i, (o, lh, rh, tp, rng) in enumerate(calls):
                    covered = set()
                    for _, _, _, _, r in calls[:i]:
                        covered.update(range(*r))
                    start = not set(range(*rng)).issubset(covered)
                    stop = not any(max(rng[0], r[0]) < min(rng[1], r[1])
                                   for _, _, _, _, r in calls[i + 1:])
                    # skip_group_check: the executing-sim group checker
                    # mis-addresses partition-based PSUM offsets (tensor rows
                    # != 16KB); data semantics are still simulated exactly.
                    nc.tensor.matmul(o, lh, rh, start=start, stop=stop,
                                     tile_position=tp, skip_group_check=True)
            if l == 0 and k > 0:
                nc.tensor.matmul(G[:, s, :], rw_sb[:, 0, s, :], h_prev[cc],
                                 start=False, stop=True,
                                 skip_group_check=True)

        S = sbB.tile([128, 4, bk], BF16, tag=f"s{cc}", bufs=3)
        if l == 0:
            # biases folded into the projection: bias-free activations; f
            # separate so f*c can start before the i,o sigmoids finish
            nc.scalar.activation(out=S[:, SLOT_G, :], in_=G[:, SLOT_G, :],
                                 func=AF.Tanh)
            nc.scalar.activation(out=S[:, SLOT_F, :], in_=G[:, SLOT_F, :],
                                 func=AF.Sigmoid)
            nc.scalar.activation(out=S[:, SLOT_I:SLOT_O + 1, :],
                                 in_=G[:, SLOT_I:SLOT_O + 1, :],
                                 func=AF.Sigmoid)
        else:
            b0 = (l - 1) * 4
            for s, fn in ((SLOT_G, AF.Tanh), (SLOT_F, AF.Sigmoid),
                          (SLOT_I, AF.Sigmoid), (SLOT_O, AF.Sigmoid)):
                nc.scalar.activation(out=S[:, s, :], in_=G[:, s, :], func=fn,
                                     bias=br_sb[:, b0 + s:b0 + s + 1])

        if k == 0:
            c = sbS.tile([128, bk], F32, tag=f"c{cc}")
            c_st[cc] = c
            nc.vector.tensor_mul(c, S[:, SLOT_I, :], S[:, SLOT_G, :])  # i*g
        else:
            c = c_st[cc]
            nc.gpsimd.tensor_mul(c, S[:, SLOT_F, :], c)            # f*c (POOL)
            u = sbB.tile([128, bk], BF16, tag=f"u{cc}")
            nc.vector.tensor_mul(u, S[:, SLOT_I, :], S[:, SLOT_G, :])  # (DVE)
            nc.gpsimd.tensor_add(c, c, u)                          # (POOL)
        return S, c

    def lstm_step_ph2(cc, l, k, S, c):
        # second phase emitted after the other chunk's phase 1 so the
        # ScalarE FIFO never head-of-line blocks on tanh(c) while the other
        # chunk's (ready) sigmoid sits behind it
        c0 = cc * bk
        cols = slice(c0, c0 + bk)
        rt = t_len - 1 - k
        Tc = sbB.tile([128, bk], BF16, tag=f"tc{cc}")
        nc.scalar.activation(out=Tc, in_=c, func=AF.Tanh)
        h = sbA.tile([128, bk], BF16, tag=f"h{cc}")
        nc.vector.tensor_mul(h, S[:, SLOT_O, :], Tc)               # h = o*tanh(c)
        h_prev[cc] = h

        # store time-ordered halves: fwd half at t=k, bwd half at t=rt
        nc.sync.dma_start(out=O[l][0:64, k, cols], in_=h[0:64, :])
        nc.sync.dma_start(out=O[l][64:128, rt, cols], in_=h[64:128, :])

    PF = min(2, t_len - 1)
    for l in range(NL):
        pend = {}
        for kk in range(PF):
            for cc in range(CHUNKS):
                pend[(cc, kk)] = issue_inp(cc, l, kk)
        for k in range(t_len):
            ph1 = {}
            for cc in range(CHUNKS):
                if k + PF < t_len:
                    pend[(cc, k + PF)] = issue_inp(cc, l, k + PF)
                ph1[cc] = lstm_step(cc, l, k, pend.pop((cc, k)))
            for cc in range(CHUNKS):
                S, c = ph1[cc]
                lstm_step_ph2(cc, l, k, S, c)

    # ---- final stage: relu + residual + LayerNorm + transpose to natural ----
    def issue_fin(cc, t):
        c0 = cc * bk
        cols = slice(c0, c0 + bk)
        o2t = sbA.tile([128, bk], BF16, tag=f"inf{cc}", bufs=4, name="o2t")
        nc.gpsimd.dma_start(out=o2t, in_=O[NL - 1][:, t, cols])
        xt = sbA.tile([9, bk], BF16, tag=f"inb{cc}", bufs=4, name="xt")
        nc.scalar.dma_start(out=xt, in_=xaug[0:9, t, cols])
        return o2t, xt

    def final_t(cc, t, o2t, xt):
        c0 = cc * bk
        relu4 = sbB.tile([128, bk], F32, tag=f"relu{cc}")
        nc.vector.tensor_scalar_max(relu4, o2t, 0.0)
        # one accumulation group for the whole bank: transpose overwrites its
        # quarter (pending-zero from the single start), residual accumulates
        psZ = ps.tile([128, 4, 128], F32, tag=f"g{cc}")
        for bi in range(4):
            bs = slice(bi * 128, (bi + 1) * 128)
            nc.tensor.matmul(psZ[:, bi, :], relu4[:, bs], ident_sb,
                             is_transpose=True, start=(bi == 0), stop=False,
                             skip_group_check=True)
            nc.tensor.matmul(psZ[:, bi, :], xt[:, bs], wres_sb,
                             start=False, stop=(bi == 3),
                             skip_group_check=True)
        # z to SBUF once on the (otherwise idle) ScalarE, stats + normalize
        # read it from SBUF on DVE/Pool
        z = sbB.tile([128, 4, 128], F32, tag=f"z{cc}")
        nc.scalar.copy(out=z, in_=psZ)
        bnst = sbB.tile([128, 4, 6], F32, tag=f"bn{cc}")
        muvar = sbB.tile([128, 4, 2], F32, tag=f"mv{cc}")
        for bi in range(4):
            nc.vector.bn_stats(bnst[:, bi, :], z[:, bi, :])
            nc.vector.bn_aggr(muvar[:, bi, :], bnst[:, bi, :])
        sd = sbB.tile([128, 4, 1], F32, tag=f"sd{cc}")
        nc.scalar.activation(out=sd, in_=muvar[:, :, 1:2], func=AF.Sqrt,
                             bias=eps_sb)
        rstd = sbB.tile([128, 4, 1], F32, tag=f"rs{cc}")
        nc.vector.reciprocal_approx_fast(rstd, sd)
        nmr = sbB.tile([128, 4, 1], F32, tag=f"nm{cc}")
        nc.vector.scalar_tensor_tensor(nmr, muvar[:, :, 0:1], -1.0, rstd,
                                       op0=OP.mult, op1=OP.mult)
        onat = sbA.tile([128, 4, 128], F32, tag=f"on{cc}", bufs=3)
        for bi in range(4):
            nc.gpsimd.tensor_scalar(onat[:, bi, :], z[:, bi, :],
                                    rstd[:, bi, :], nmr[:, bi, :],
                                    op0=OP.mult, op1=OP.add)
            if apply_gb:
                nc.gpsimd.tensor_mul(onat[:, bi, :], onat[:, bi, :], gamma_sb)
                nc.gpsimd.tensor_add(onat[:, bi, :], onat[:, bi, :], beta_sb)
        oap = out_d[c0:c0 + bk, t, :].rearrange("(b p) f -> p b f", p=128)
        nc.sync.dma_start(out=oap, in_=onat)

    fpend = {}
    for tt in range(PF):
        for cc in range(CHUNKS):
            fpend[(cc, tt)] = issue_fin(cc, tt)
    for t in range(t_len):
        for cc in range(CHUNKS):
            if t + PF < t_len:
                fpend[(cc, t + PF)] = issue_fin(cc, t + PF)
            o2t, xt = fpend.pop((cc, t))
            final_t(cc, t, o2t, xt)


def build(apply_gb=False, bc=BC, t_len=T, num_devices=NCORES):
    nc = bacc.Bacc("TRN2", target_bir_lowering=False, debug=False,
                   num_devices=num_devices)
    D = {}

    def inp(name, shape, dtype=F32):
        D[name] = nc.dram_tensor(name, shape, dtype, kind="ExternalInput").ap()

    inp("xaug", [18, t_len, bc], BF16)
    inp("rw", [128, NL, 4, 128], BF16)
    inp("pw", [128, NL - 1, 4, 2, 64], BF16)
    inp("l0w", [18, 4, 128], BF16)
    inp("br", [128, (NL - 1) * 4])
    inp("wres", [9, 128], BF16)
    inp("ident", [128, 128])
    if apply_gb:
        inp("gammab", [128, 128])
        inp("betab", [128, 128])
    for i in range(NL):
        D[f"o{i}"] = nc.dram_tensor(f"o{i}", [128, t_len, bc], BF16).ap()
    D["out"] = nc.dram_tensor("out", [bc, t_len, 128], F32,
                              kind="ExternalOutput").ap()

    with tile.TileContext(nc) as tc:
        with ExitStack() as ctx:
            _emit(nc, tc, ctx, D, apply_gb, bc, t_len)
    nc.compile()
    return nc


_BUILD_CACHE = {}


def make_in_maps(inputs, apply_gb):
    ln_gamma = np.asarray(inputs["ln_gamma"], np.float32)
    ln_beta = np.asarray(inputs["ln_beta"], np.float32)
    shared, xaug_cores = _host_prep(
        inputs["x"], inputs["w_ih"], inputs["w_hh"], inputs["b_ih"],
        inputs["b_hh"], inputs["w_res"], inputs["b_res"], NCORES, BC)
    in_maps = []
    for c in range(NCORES):
        m = dict(shared)
        m["xaug"] = xaug_cores[c]
        if apply_gb:
            m["gammab"] = np.ascontiguousarray(
                np.broadcast_to(ln_gamma, (128, 128)).astype(np.float32))
            m["betab"] = np.ascontiguousarray(
                np.broadcast_to(ln_beta, (128, 128)).astype(np.float32))
        in_maps.append(m)
    return in_maps


def kernel(x, w_ih, w_hh, b_ih, b_hh, w_res, b_res, ln_gamma, ln_beta):
    ln_gamma = np.asarray(ln_gamma, np.float32)
    ln_beta = np.asarray(ln_beta, np.float32)
    apply_gb = not (np.all(ln_gamma == 1.0) and np.all(ln_beta == 0.0))

    if apply_gb not in _BUILD_CACHE:
        _BUILD_CACHE[apply_gb] = build(apply_gb)
    nc = _BUILD_CACHE[apply_gb]

    inputs = dict(x=x, w_ih=w_ih, w_hh=w_hh, b_ih=b_ih, b_hh=b_hh,
                  w_res=w_res, b_res=b_res, ln_gamma=ln_gamma, ln_beta=ln_beta)
    in_maps = make_in_maps(inputs, apply_gb)

    res = run_bass_kernel_spmd(nc, in_maps, core_ids=list(range(NCORES)))
    out = np.concatenate([res.results[c]["out"] for c in range(NCORES)], axis=0)
    return np.ascontiguousarray(out.astype(np.float32))


# revision 20
# speedup vs baseline: 2.0198x; 1.0163x over previous
"""Trainium2 Bass kernel for a 3-layer BiLSTM + ReLU + residual + LayerNorm.

Strategy (pure data parallel over 8 cores, 1024 batch rows per core):
  * "Transposed" layout on-chip: features on SBUF partitions, batch on the
    free dim.  Both directions fused on partitions (fwd = 0:64, bwd = 64:128)
    so every engine op runs with all 128 lanes busy.
  * All matmuls in bf16 (4x cheaper per row than fp32 on the PE): per
    timestep, per gate, a col-tiled input-projection pair plus one 128x128
    block-diagonal recurrent matmul accumulate into one PSUM bank; the four
    gates share a [128, 4, bk] PSUM tensor ordered (i, f, o, g).
  * Layer 0 folds both time directions AND the biases into a single K=18
    augmented input ([x_t; 1; x_rt; 1]), so its activations need no bias:
    one sigmoid over the (i,f,o) span + one tanh for g.
  * Layers 1-2 keep per-gate activations with per-partition bias.
  * Elementwise rebalance: i*g and h=o*tanh(c) on DVE in bf16, f*c on the
    Pool engine, c accumulates in fp32 on DVE.
  * Final stage: bf16 PE transpose to natural layout + K=9 residual matmul
    into one PSUM bank, LayerNorm stats via one bn_stats + four bn_aggr on
    DVE, rstd via the approx reciprocal, normalization on Pool directly from
    PSUM, DMA out natural-layout fp32.
  * Hot-loop input DMAs issue from the Pool queue (cheap dispatch), output
    stores from the SP queue.
"""

from contextlib import ExitStack

import numpy as np
import ml_dtypes

import concourse.bacc as bacc
import concourse.tile as tile
from concourse import mybir
from concourse.bass_utils import run_bass_kernel_spmd

F32 = mybir.dt.float32
BF16 = mybir.dt.bfloat16
AF = mybir.ActivationFunctionType
OP = mybir.AluOpType

NCORES = 8
BC = 1024               # batch rows per core
CHUNKS = 2
T = 64
H = 64
NL = 3
D2 = 2 * H              # 128
LN_EPS = 1e-5

# gate order in PyTorch weights: i, f, g, o (rows g*H:(g+1)*H of w_ih/w_hh)
# on-chip slot order: g, f, i, o -- g first so its tanh clears the ACT queue
# early, f second so the Pool f*c starts before the i/o sigmoids; i,o
# adjacent so layer-0 can sigmoid them as one span
PT2SLOT = {0: 2, 1: 1, 2: 0, 3: 3}   # pytorch gate idx -> PSUM slot
SLOT_G, SLOT_F, SLOT_I, SLOT_O = 0, 1, 2, 3


def _host_prep(x, w_ih, w_hh, b_ih, b_hh, w_res, b_res, ncores, bc):
    """Matmul-ready bf16 weight layouts (shared across cores) + per-core
    inputs."""
    x = np.asarray(x, np.float32)
    w_ih = np.asarray(w_ih, np.float32)
    w_hh = np.asarray(w_hh, np.float32)
    bias = np.asarray(b_ih, np.float32) + np.asarray(b_hh, np.float32)  # (NL,2,4H)
    w_res = np.asarray(w_res, np.float32)
    b_res = np.asarray(b_res, np.float32)
    t_len = x.shape[1]

    # Recurrent lhsT, K-major: rw[k, l, slot, m] (block-diagonal over dirs)
    rw = np.zeros((128, NL, 4, 128), np.float32)
    for l in range(NL):
        for g in range(4):
            s = PT2SLOT[g]
            gs = slice(g * H, (g + 1) * H)
            rw[0:64, l, s, 0:64] = w_hh[l, 0, gs, :].T
            rw[64:128, l, s, 64:128] = w_hh[l, 1, gs, :].T
    rw = rw.astype(ml_dtypes.bfloat16)

    # Input-projection lhsT for layers 1,2 (bf16): pw[k, l-1, slot, d, m]
    pw = np.zeros((128, NL - 1, 4, 2, 64), np.float32)
    for l in (1, 2):
        for g in range(4):
            s = PT2SLOT[g]
            gs = slice(g * H, (g + 1) * H)
            for d in range(2):
                pw[:, l - 1, s, d, :] = w_ih[l, d, gs, :].T
    pw = pw.astype(ml_dtypes.bfloat16)

    # Layer-0 fused fwd+bwd projection lhsT with biases on the ones-rows:
    # K rows 0:8 = x_t, row 8 = 1, rows 9:17 = x_rt, row 17 = 1.
    l0w = np.zeros((18, 4, 128), np.float32)
    for g in range(4):
        s = PT2SLOT[g]
        gs = slice(g * H, (g + 1) * H)
        l0w[0:8, s, 0:64] = w_ih[0, 0, gs, 0:8].T
        l0w[8, s, 0:64] = bias[0, 0, gs]
        l0w[9:17, s, 64:128] = w_ih[0, 1, gs, 0:8].T
        l0w[17, s, 64:128] = bias[0, 1, gs]
    l0w = l0w.astype(ml_dtypes.bfloat16)

    # per-partition gate biases, layers 1,2 (fused dirs): br[p, (l-1)*4+slot]
    br = np.zeros((128, (NL - 1) * 4), np.float32)
    for l in (1, 2):
        for g in range(4):
            s = PT2SLOT[g]
            gs = slice(g * H, (g + 1) * H)
            br[0:64, (l - 1) * 4 + s] = bias[l, 0, gs]
            br[64:128, (l - 1) * 4 + s] = bias[l, 1, gs]

    # residual rhs: wres[k, f] = w_res[f, k], row 8 = b_res
    wres = np.zeros((9, 128), np.float32)
    wres[0:8, :] = w_res.T
    wres[8, :] = b_res
    wres = wres.astype(ml_dtypes.bfloat16)

    ident = np.eye(128, dtype=np.float32)

    # Per-core transposed-augmented input: xaug[k, t, b] with both time
    # directions stacked: rows 0:8 = x_t, 8 = 1, 9:17 = x_{T-1-t}, 17 = 1
    xaug_cores = []
    for c in range(ncores):
        xc = x[c * bc:(c + 1) * bc]              # (bc, T, 8)
        xa = np.empty((18, t_len, bc), np.float32)
        xa[0:8] = xc.transpose(2, 1, 0)
        xa[8] = 1.0
        xa[9:17] = xc[:, ::-1].transpose(2, 1, 0)
        xa[17] = 1.0
        xaug_cores.append(xa.astype(ml_dtypes.bfloat16))

    shared = dict(rw=rw, pw=pw, l0w=l0w, br=br, wres=wres, ident=ident)
    return shared, xaug_cores


def _emit(nc, tc, ctx, D, apply_gb, bc, t_len):
    bk = bc // CHUNKS

    sbC = ctx.enter_context(tc.tile_pool(name="consts", bufs=1))
    sbA = ctx.enter_context(tc.tile_pool(name="workA", bufs=3))
    sbB = ctx.enter_context(tc.tile_pool(name="workB", bufs=2))
    sbS = ctx.enter_context(tc.tile_pool(name="state", bufs=1))
    ps = ctx.enter_context(tc.tile_pool(name="ps", bufs=1, space="PSUM"))

    def const_tile(shape, dtype, key):
        t = sbC.tile(shape, dtype, name=f"c_{key}", tag=f"c_{key}")
        nc.sync.dma_start(out=t, in_=D[key])
        return t

    rw_sb = const_tile([128, NL, 4, 128], BF16, "rw")
    pw_sb = const_tile([128, NL - 1, 4, 2, 64], BF16, "pw")
    l0w_sb = const_tile([18, 4, 128], BF16, "l0w")
    br_sb = const_tile([128, (NL - 1) * 4], F32, "br")
    wres_sb = const_tile([9, 128], BF16, "wres")
    ident_sb = const_tile([128, 128], F32, "ident")
    gamma_sb = beta_sb = None
    if apply_gb:
        gamma_sb = const_tile([128, 128], F32, "gammab")
        beta_sb = const_tile([128, 128], F32, "betab")
    eps_sb = sbC.tile([128, 1], F32)
    nc.vector.memset(eps_sb, LN_EPS)

    O = [D[f"o{i}"] for i in range(NL)]
    xaug = D["xaug"]
    out_d = D["out"]

    h_prev = [None] * CHUNKS
    c_st = [None] * CHUNKS

    def issue_inp(cc, l, k):
        c0 = cc * bk
        cols = slice(c0, c0 + bk)
        rt = t_len - 1 - k
        if l == 0:
            xin = sbA.tile([18, bk], BF16, tag=f"inf{cc}", bufs=4, name="xin")
            nc.gpsimd.dma_start(out=xin, in_=xaug[:, k, cols])
            return (xin,)
        # both time slices (k and rt) in one strided DMA; half order follows
        # the slice direction
        t0, t1 = (k, rt) if k < rt else (rt, k)
        inp2 = sbA.tile([128, 2, bk], BF16, tag=f"inf{cc}", bufs=4,
                        name="inp2")
        nc.gpsimd.dma_start(out=inp2,
                            in_=O[l - 1][:, t0:t1 + 1:t1 - t0, cols])
        if k < rt:
            return (inp2[:, 0, :], inp2[:, 1, :])
        return (inp2[:, 1, :], inp2[:, 0, :])

    def lstm_step(cc, l, k, inps):
        G = ps.tile([128, 4, bk], F32, tag=f"g{cc}")

        for s in range(4):
            if l == 0:
                (xin,) = inps
                # fused fwd+bwd proj with bias rows, K=18
                nc.tensor.matmul(G[:, s, :], l0w_sb[:, s, :], xin,
                                 start=True, stop=(k == 0),
                                 skip_group_check=True)
            else:
                inp_f, inp_b = inps
                calls = [
                    (G[0:64, s, :], pw_sb[:, l - 1, s, 0, :], inp_f,
                     (0, 0), (0, 64)),
                    (G[64:128, s, :], pw_sb[:, l - 1, s, 1, :], inp_b,
                     (0, 64), (64, 128)),
                ]
                if k > 0:
                    calls.append((G[:, s, :], rw_sb[:, l, s, :], h_prev[cc],
                                  None, (0, 128)))
                n = len(calls)
                for i, (o, lh, rh, tp, rng) in enumerate(calls):
                    covered = set()
                    for _, _, _, _, r in calls[:i]:
                        covered.update(range(*r))
                    start = not set(range(*rng)).issubset(covered)
                    stop = not any(max(rng[0], r[0]) < min(rng[1], r[1])
                                   for _, _, _, _, r in calls[i + 1:])
                    # skip_group_check: the executing-sim group checker
                    # mis-addresses partition-based PSUM offsets (tensor rows
                    # != 16KB); data semantics are still simulated exactly.
                    nc.tensor.matmul(o, lh, rh, start=start, stop=stop,
                                     tile_position=tp, skip_group_check=True)
            if l == 0 and k > 0:
                nc.tensor.matmul(G[:, s, :], rw_sb[:, 0, s, :], h_prev[cc],
                                 start=False, stop=True,
                                 skip_group_check=True)

        # the o sigmoid is off the c-critical-path; deferring it for the
        # second chunk lets the first chunk's tanh(c) enter the ACT queue
        # earlier, hiding the c-chain and h->rec latencies of both chunks
        defer_o = (cc == 1)
        S = sbB.tile([128, 4, bk], BF16, tag=f"s{cc}", bufs=3)

        def sig_o():
            if l == 0:
                nc.scalar.activation(out=S[:, SLOT_O, :], in_=G[:, SLOT_O, :],
                                     func=AF.Sigmoid)
            else:
                b0 = (l - 1) * 4
                nc.scalar.activation(out=S[:, SLOT_O, :], in_=G[:, SLOT_O, :],
                                     func=AF.Sigmoid,
                                     bias=br_sb[:, b0 + SLOT_O:b0 + SLOT_O + 1])

        if l == 0:
            # biases folded into the projection: bias-free activations; f
            # separate so f*c can start before the i,o sigmoids finish
            nc.scalar.activation(out=S[:, SLOT_G, :], in_=G[:, SLOT_G, :],
                                 func=AF.Tanh)
            nc.scalar.activation(out=S[:, SLOT_F, :], in_=G[:, SLOT_F, :],
                                 func=AF.Sigmoid)
            if defer_o:
                nc.scalar.activation(out=S[:, SLOT_I, :], in_=G[:, SLOT_I, :],
                                     func=AF.Sigmoid)
            else:
                nc.scalar.activation(out=S[:, SLOT_I:SLOT_O + 1, :],
                                     in_=G[:, SLOT_I:SLOT_O + 1, :],
                                     func=AF.Sigmoid)
        else:
            b0 = (l - 1) * 4
            gates = [(SLOT_G, AF.Tanh), (SLOT_F, AF.Sigmoid),
                     (SLOT_I, AF.Sigmoid)]
            if not defer_o:
                gates.append((SLOT_O, AF.Sigmoid))
            for s, fn in gates:
                nc.scalar.activation(out=S[:, s, :], in_=G[:, s, :], func=fn,
                                     bias=br_sb[:, b0 + s:b0 + s + 1])

        if k == 0:
            c = sbS.tile([128, bk], F32, tag=f"c{cc}")
            c_st[cc] = c
            nc.vector.tensor_mul(c, S[:, SLOT_I, :], S[:, SLOT_G, :])  # i*g
        else:
            c = c_st[cc]
            nc.gpsimd.tensor_mul(c, S[:, SLOT_F, :], c)            # f*c (POOL)
            u = sbB.tile([128, bk], BF16, tag=f"u{cc}")
            nc.vector.tensor_mul(u, S[:, SLOT_I, :], S[:, SLOT_G, :])  # (DVE)
            nc.gpsimd.tensor_add(c, c, u)                          # (POOL)
        return S, c, (sig_o if defer_o else None)

    def lstm_step_ph2(cc, l, k, S, c, sig_o):
        if sig_o is not None:
            sig_o()
        # second phase emitted after the other chunk's phase 1 so the
        # ScalarE FIFO never head-of-line blocks on tanh(c) while the other
        # chunk's (ready) sigmoid sits behind it
        c0 = cc * bk
        cols = slice(c0, c0 + bk)
        rt = t_len - 1 - k
        Tc = sbB.tile([128, bk], BF16, tag=f"tc{cc}")
        nc.scalar.activation(out=Tc, in_=c, func=AF.Tanh)
        h = sbA.tile([128, bk], BF16, tag=f"h{cc}")
        nc.vector.tensor_mul(h, S[:, SLOT_O, :], Tc)               # h = o*tanh(c)
        h_prev[cc] = h

        # store time-ordered halves: fwd half at t=k, bwd half at t=rt
        nc.sync.dma_start(out=O[l][0:64, k, cols], in_=h[0:64, :])
        nc.sync.dma_start(out=O[l][64:128, rt, cols], in_=h[64:128, :])

    PF = min(2, t_len - 1)
    for l in range(NL):
        pend = {}
        for kk in range(PF):
            for cc in range(CHUNKS):
                pend[(cc, kk)] = issue_inp(cc, l, kk)
        for k in range(t_len):
            ph1 = {}
            for cc in range(CHUNKS):
                if k + PF < t_len:
                    pend[(cc, k + PF)] = issue_inp(cc, l, k + PF)
                ph1[cc] = lstm_step(cc, l, k, pend.pop((cc, k)))
            for cc in range(CHUNKS):
                S, c, sig_o = ph1[cc]
                lstm_step_ph2(cc, l, k, S, c, sig_o)

    # ---- final stage: relu + residual + LayerNorm + transpose to natural ----
    def issue_fin(cc, t):
        c0 = cc * bk
        cols = slice(c0, c0 + bk)
        o2t = sbA.tile([128, bk], BF16, tag=f"inf{cc}", bufs=4, name="o2t")
        nc.gpsimd.dma_start(out=o2t, in_=O[NL - 1][:, t, cols])
        xt = sbA.tile([9, bk], BF16, tag=f"inb{cc}", bufs=4, name="xt")
        nc.scalar.dma_start(out=xt, in_=xaug[0:9, t, cols])
        return o2t, xt

    def final_t(cc, t, o2t, xt):
        c0 = cc * bk
        relu4 = sbB.tile([128, bk], F32, tag=f"relu{cc}")
        nc.vector.tensor_scalar_max(relu4, o2t, 0.0)
        # one accumulation group for the whole bank: transpose overwrites its
        # quarter (pending-zero from the single start), residual accumulates
        psZ = ps.tile([128, 4, 128], F32, tag=f"g{cc}")
        for bi in range(4):
            bs = slice(bi * 128, (bi + 1) * 128)
            nc.tensor.matmul(psZ[:, bi, :], relu4[:, bs], ident_sb,
                             is_transpose=True, start=(bi == 0), stop=False,
                             skip_group_check=True)
            nc.tensor.matmul(psZ[:, bi, :], xt[:, bs], wres_sb,
                             start=False, stop=(bi == 3),
                             skip_group_check=True)
        # z to SBUF once on the (otherwise idle) ScalarE, stats + normalize
        # read it from SBUF on DVE/Pool
        z = sbB.tile([128, 4, 128], F32, tag=f"z{cc}")
        nc.scalar.copy(out=z, in_=psZ)
        bnst = sbB.tile([128, 4, 6], F32, tag=f"bn{cc}")
        muvar = sbB.tile([128, 4, 2], F32, tag=f"mv{cc}")
        for bi in range(4):
            nc.vector.bn_stats(bnst[:, bi, :], z[:, bi, :])
            nc.vector.bn_aggr(muvar[:, bi, :], bnst[:, bi, :])
        sd = sbB.tile([128, 4, 1], F32, tag=f"sd{cc}")
        nc.scalar.activation(out=sd, in_=muvar[:, :, 1:2], func=AF.Sqrt,
                             bias=eps_sb)
        rstd = sbB.tile([128, 4, 1], F32, tag=f"rs{cc}")
        nc.vector.reciprocal_approx_fast(rstd, sd)
        nmr = sbB.tile([128, 4, 1], F32, tag=f"nm{cc}")
        nc.vector.scalar_tensor_tensor(nmr, muvar[:, :, 0:1], -1.0, rstd,
                                       op0=OP.mult, op1=OP.mult)
        onat = sbA.tile([128, 4, 128], F32, tag=f"on{cc}", bufs=3)
        for bi in range(4):
            nc.gpsimd.tensor_scalar(onat[:, bi, :], z[:, bi, :],
                                    rstd[:, bi, :], nmr[:, bi, :],
                                    op0=OP.mult, op1=OP.add)
            if apply_gb:
                nc.gpsimd.tensor_mul(onat[:, bi, :], onat[:, bi, :], gamma_sb)
                nc.gpsimd.tensor_add(onat[:, bi, :], onat[:, bi, :], beta_sb)
        oap = out_d[c0:c0 + bk, t, :].rearrange("(b p) f -> p b f", p=128)
        nc.sync.dma_start(out=oap, in_=onat)

    fpend = {}
    for tt in range(PF):
        for cc in range(CHUNKS):
            fpend[(cc, tt)] = issue_fin(cc, tt)
    for t in range(t_len):
        for cc in range(CHUNKS):
            if t + PF < t_len:
                fpend[(cc, t + PF)] = issue_fin(cc, t + PF)
            o2t, xt = fpend.pop((cc, t))
            final_t(cc, t, o2t, xt)


def build(apply_gb=False, bc=BC, t_len=T, num_devices=NCORES):
    nc = bacc.Bacc("TRN2", target_bir_lowering=False, debug=False,
                   num_devices=num_devices)
    D = {}

    def inp(name, shape, dtype=F32):
        D[name] = nc.dram_tensor(name, shape, dtype, kind="ExternalInput").ap()

    inp("xaug", [18, t_len, bc], BF16)
    inp("rw", [128, NL, 4, 128], BF16)
    inp("pw", [128, NL - 1, 4, 2, 64], BF16)
    inp("l0w", [18, 4, 128], BF16)
    inp("br", [128, (NL - 1) * 4])
    inp("wres", [9, 128], BF16)
    inp("ident", [128, 128])
    if apply_gb:
        inp("gammab", [128, 128])
        inp("betab", [128, 128])
    for i in range(NL):
        D[f"o{i}"] = nc.dram_tensor(f"o{i}", [128, t_len, bc], BF16).ap()
    D["out"] = nc.dram_tensor("out", [bc, t_len, 128], F32,
                              kind="ExternalOutput").ap()

    with tile.TileContext(nc) as tc:
        with ExitStack() as ctx:
            _emit(nc, tc, ctx, D, apply_gb, bc, t_len)
    nc.compile()
    return nc


_BUILD_CACHE = {}


def make_in_maps(inputs, apply_gb):
    ln_gamma = np.asarray(inputs["ln_gamma"], np.float32)
    ln_beta = np.asarray(inputs["ln_beta"], np.float32)
    shared, xaug_cores = _host_prep(
        inputs["x"], inputs["w_ih"], inputs["w_hh"], inputs["b_ih"],
        inputs["b_hh"], inputs["w_res"], inputs["b_res"], NCORES, BC)
    in_maps = []
    for c in range(NCORES):
        m = dict(shared)
        m["xaug"] = xaug_cores[c]
        if apply_gb:
            m["gammab"] = np.ascontiguousarray(
                np.broadcast_to(ln_gamma, (128, 128)).astype(np.float32))
            m["betab"] = np.ascontiguousarray(
                np.broadcast_to(ln_beta, (128, 128)).astype(np.float32))
        in_maps.append(m)
    return in_maps


def kernel(x, w_ih, w_hh, b_ih, b_hh, w_res, b_res, ln_gamma, ln_beta):
    ln_gamma = np.asarray(ln_gamma, np.float32)
    ln_beta = np.asarray(ln_beta, np.float32)
    apply_gb = not (np.all(ln_gamma == 1.0) and np.all(ln_beta == 0.0))

    if apply_gb not in _BUILD_CACHE:
        _BUILD_CACHE[apply_gb] = build(apply_gb)
    nc = _BUILD_CACHE[apply_gb]

    inputs = dict(x=x, w_ih=w_ih, w_hh=w_hh, b_ih=b_ih, b_hh=b_hh,
                  w_res=w_res, b_res=b_res, ln_gamma=ln_gamma, ln_beta=ln_beta)
    in_maps = make_in_maps(inputs, apply_gb)

    res = run_bass_kernel_spmd(nc, in_maps, core_ids=list(range(NCORES)))
    out = np.concatenate([res.results[c]["out"] for c in range(NCORES)], axis=0)
    return np.ascontiguousarray(out.astype(np.float32))


# revision 25
# speedup vs baseline: 2.0548x; 1.0173x over previous
"""Trainium2 Bass kernel for a 3-layer BiLSTM + ReLU + residual + LayerNorm.

Strategy (pure data parallel over 8 cores, 1024 batch rows per core):
  * "Transposed" layout on-chip: features on SBUF partitions, batch on the
    free dim.  Both directions fused on partitions (fwd = 0:64, bwd = 64:128)
    so every engine op runs with all 128 lanes busy.
  * All matmuls in bf16 (4x cheaper per row than fp32 on the PE): per
    timestep, per gate, a col-tiled input-projection pair plus one 128x128
    block-diagonal recurrent matmul accumulate into one PSUM bank; the four
    gates share a [128, 4, bk] PSUM tensor ordered (i, f, o, g).
  * Layer 0 folds both time directions AND the biases into a single K=18
    augmented input ([x_t; 1; x_rt; 1]), so its activations need no bias:
    one sigmoid over the (i,f,o) span + one tanh for g.
  * Layers 1-2 keep per-gate activations with per-partition bias.
  * Elementwise rebalance: i*g and h=o*tanh(c) on DVE in bf16, f*c on the
    Pool engine, c accumulates in fp32 on DVE.
  * Final stage: bf16 PE transpose to natural layout + K=9 residual matmul
    into one PSUM bank, LayerNorm stats via one bn_stats + four bn_aggr on
    DVE, rstd via the approx reciprocal, normalization on Pool directly from
    PSUM, DMA out natural-layout fp32.
  * Hot-loop input DMAs issue from the Pool queue (cheap dispatch), output
    stores from the SP queue.
"""

from contextlib import ExitStack

import numpy as np
import ml_dtypes

import concourse.bacc as bacc
import concourse.tile as tile
from concourse import mybir
from concourse.bass_utils import run_bass_kernel_spmd

F32 = mybir.dt.float32
BF16 = mybir.dt.bfloat16
AF = mybir.ActivationFunctionType
OP = mybir.AluOpType

NCORES = 8
BC = 1024               # batch rows per core
CHUNKS = 2
T = 64
H = 64
NL = 3
D2 = 2 * H              # 128
LN_EPS = 1e-5

# gate order in PyTorch weights: i, f, g, o (rows g*H:(g+1)*H of w_ih/w_hh)
# on-chip slot order: g, f, i, o -- g first so its tanh clears the ACT queue
# early, f second so the Pool f*c starts before the i/o sigmoids; i,o
# adjacent so layer-0 can sigmoid them as one span
PT2SLOT = {0: 2, 1: 1, 2: 0, 3: 3}   # pytorch gate idx -> PSUM slot
SLOT_G, SLOT_F, SLOT_I, SLOT_O = 0, 1, 2, 3


def _host_prep(x, w_ih, w_hh, b_ih, b_hh, w_res, b_res, ncores, bc):
    """Matmul-ready bf16 weight layouts (shared across cores) + per-core
    inputs."""
    x = np.asarray(x, np.float32)
    w_ih = np.asarray(w_ih, np.float32)
    w_hh = np.asarray(w_hh, np.float32)
    bias = np.asarray(b_ih, np.float32) + np.asarray(b_hh, np.float32)  # (NL,2,4H)
    w_res = np.asarray(w_res, np.float32)
    b_res = np.asarray(b_res, np.float32)
    t_len = x.shape[1]

    # Recurrent lhsT, K-major: rw[k, l, slot, m] (block-diagonal over dirs)
    rw = np.zeros((128, NL, 4, 128), np.float32)
    for l in range(NL):
        for g in range(4):
            s = PT2SLOT[g]
            gs = slice(g * H, (g + 1) * H)
            rw[0:64, l, s, 0:64] = w_hh[l, 0, gs, :].T
            rw[64:128, l, s, 64:128] = w_hh[l, 1, gs, :].T
    rw = rw.astype(ml_dtypes.bfloat16)

    # Input-projection lhsT for layers 1,2 (bf16): pw[k, l-1, slot, d, m]
    pw = np.zeros((128, NL - 1, 4, 2, 64), np.float32)
    for l in (1, 2):
        for g in range(4):
            s = PT2SLOT[g]
            gs = slice(g * H, (g + 1) * H)
            for d in range(2):
                pw[:, l - 1, s, d, :] = w_ih[l, d, gs, :].T
    pw = pw.astype(ml_dtypes.bfloat16)

    # Layer-0 fused fwd+bwd projection lhsT with biases on the ones-rows:
    # K rows 0:8 = x_t, row 8 = 1, rows 9:17 = x_rt, row 17 = 1.
    l0w = np.zeros((18, 4, 128), np.float32)
    for g in range(4):
        s = PT2SLOT[g]
        gs = slice(g * H, (g + 1) * H)
        l0w[0:8, s, 0:64] = w_ih[0, 0, gs, 0:8].T
        l0w[8, s, 0:64] = bias[0, 0, gs]
        l0w[9:17, s, 64:128] = w_ih[0, 1, gs, 0:8].T
        l0w[17, s, 64:128] = bias[0, 1, gs]
    l0w = l0w.astype(ml_dtypes.bfloat16)

    # per-partition gate biases, layers 1,2 (fused dirs): br[p, (l-1)*4+slot]
    br = np.zeros((128, (NL - 1) * 4), np.float32)
    for l in (1, 2):
        for g in range(4):
            s = PT2SLOT[g]
            gs = slice(g * H, (g + 1) * H)
            br[0:64, (l - 1) * 4 + s] = bias[l, 0, gs]
            br[64:128, (l - 1) * 4 + s] = bias[l, 1, gs]

    # residual rhs: wres[k, f] = w_res[f, k], row 8 = b_res
    wres = np.zeros((9, 128), np.float32)
    wres[0:8, :] = w_res.T
    wres[8, :] = b_res
    wres = wres.astype(ml_dtypes.bfloat16)

    ident = np.eye(128, dtype=np.float32)

    # Per-core transposed-augmented input: xaug[k, t, b] with both time
    # directions stacked: rows 0:8 = x_t, 8 = 1, 9:17 = x_{T-1-t}, 17 = 1
    xaug_cores = []
    for c in range(ncores):
        xc = x[c * bc:(c + 1) * bc]              # (bc, T, 8)
        xa = np.empty((18, t_len, bc), np.float32)
        xa[0:8] = xc.transpose(2, 1, 0)
        xa[8] = 1.0
        xa[9:17] = xc[:, ::-1].transpose(2, 1, 0)
        xa[17] = 1.0
        xaug_cores.append(xa.astype(ml_dtypes.bfloat16))

    shared = dict(rw=rw, pw=pw, l0w=l0w, br=br, wres=wres, ident=ident)
    return shared, xaug_cores


def _emit(nc, tc, ctx, D, apply_gb, bc, t_len):
    bk = bc // CHUNKS

    sbC = ctx.enter_context(tc.tile_pool(name="consts", bufs=1))
    sbA = ctx.enter_context(tc.tile_pool(name="workA", bufs=3))
    sbB = ctx.enter_context(tc.tile_pool(name="workB", bufs=2))
    sbS = ctx.enter_context(tc.tile_pool(name="state", bufs=1))
    ps = ctx.enter_context(tc.tile_pool(name="ps", bufs=1, space="PSUM"))

    def const_tile(shape, dtype, key):
        t = sbC.tile(shape, dtype, name=f"c_{key}", tag=f"c_{key}")
        nc.sync.dma_start(out=t, in_=D[key])
        return t

    rw_sb = const_tile([128, NL, 4, 128], BF16, "rw")
    pw_sb = const_tile([128, NL - 1, 4, 2, 64], BF16, "pw")
    l0w_sb = const_tile([18, 4, 128], BF16, "l0w")
    br_sb = const_tile([128, (NL - 1) * 4], F32, "br")
    wres_sb = const_tile([9, 128], BF16, "wres")
    ident_sb = const_tile([128, 128], F32, "ident")
    gamma_sb = beta_sb = None
    if apply_gb:
        gamma_sb = const_tile([128, 128], F32, "gammab")
        beta_sb = const_tile([128, 128], F32, "betab")
    eps_sb = sbC.tile([128, 1], F32)
    nc.vector.memset(eps_sb, LN_EPS)

    O = [D[f"o{i}"] for i in range(NL)]
    xaug = D["xaug"]
    out_d = D["out"]

    h_prev = [None] * CHUNKS
    c_st = [None] * CHUNKS

    def issue_inp(cc, l, k):
        c0 = cc * bk
        cols = slice(c0, c0 + bk)
        rt = t_len - 1 - k
        if l == 0:
            xin = sbA.tile([18, bk], BF16, tag=f"inf{cc}", bufs=4, name="xin")
            nc.gpsimd.dma_start(out=xin, in_=xaug[:, k, cols])
            return (xin,)
        # both time slices (k and rt) in one strided DMA; half order follows
        # the slice direction
        t0, t1 = (k, rt) if k < rt else (rt, k)
        inp2 = sbA.tile([128, 2, bk], BF16, tag=f"inf{cc}", bufs=4,
                        name="inp2")
        nc.gpsimd.dma_start(out=inp2,
                            in_=O[l - 1][:, t0:t1 + 1:t1 - t0, cols])
        if k < rt:
            return (inp2[:, 0, :], inp2[:, 1, :])
        return (inp2[:, 1, :], inp2[:, 0, :])

    def lstm_step(cc, l, k, inps):
        G = ps.tile([128, 4, bk], F32, tag=f"g{cc}")

        for s in range(4):
            if l == 0:
                (xin,) = inps
                # fused fwd+bwd proj with bias rows, K=18
                nc.tensor.matmul(G[:, s, :], l0w_sb[:, s, :], xin,
                                 start=True, stop=(k == 0),
                                 skip_group_check=True)
            else:
                inp_f, inp_b = inps
                calls = [
                    (G[0:64, s, :], pw_sb[:, l - 1, s, 0, :], inp_f,
                     (0, 0), (0, 64)),
                    (G[64:128, s, :], pw_sb[:, l - 1, s, 1, :], inp_b,
                     (0, 64), (64, 128)),
                ]
                if k > 0:
                    calls.append((G[:, s, :], rw_sb[:, l, s, :], h_prev[cc],
                                  None, (0, 128)))
                n = len(calls)
                for i, (o, lh, rh, tp, rng) in enumerate(calls):
                    covered = set()
                    for _, _, _, _, r in calls[:i]:
                        covered.update(range(*r))
                    start = not set(range(*rng)).issubset(covered)
                    stop = not any(max(rng[0], r[0]) < min(rng[1], r[1])
                                   for _, _, _, _, r in calls[i + 1:])
                    # skip_group_check: the executing-sim group checker
                    # mis-addresses partition-based PSUM offsets (tensor rows
                    # != 16KB); data semantics are still simulated exactly.
                    nc.tensor.matmul(o, lh, rh, start=start, stop=stop,
                                     tile_position=tp, skip_group_check=True)
            if l == 0 and k > 0:
                nc.tensor.matmul(G[:, s, :], rw_sb[:, 0, s, :], h_prev[cc],
                                 start=False, stop=True,
                                 skip_group_check=True)

        # the o sigmoid is off the c-critical-path; deferring it for the
        # second chunk lets the first chunk's tanh(c) enter the ACT queue
        # earlier, hiding the c-chain and h->rec latencies of both chunks
        defer_o = (cc == 1)
        S = sbB.tile([128, 4, bk], BF16, tag=f"s{cc}", bufs=3)

        def sig_o():
            if l == 0:
                nc.scalar.activation(out=S[:, SLOT_O, :], in_=G[:, SLOT_O, :],
                                     func=AF.Sigmoid)
            else:
                b0 = (l - 1) * 4
                nc.scalar.activation(out=S[:, SLOT_O, :], in_=G[:, SLOT_O, :],
                                     func=AF.Sigmoid,
                                     bias=br_sb[:, b0 + SLOT_O:b0 + SLOT_O + 1])

        if l == 0:
            # biases folded into the projection: bias-free activations
            nc.scalar.activation(out=S[:, SLOT_G, :], in_=G[:, SLOT_G, :],
                                 func=AF.Tanh)
            if defer_o:
                # f separate so f*c starts early; o deferred to ph2
                nc.scalar.activation(out=S[:, SLOT_F, :], in_=G[:, SLOT_F, :],
                                     func=AF.Sigmoid)
                nc.scalar.activation(out=S[:, SLOT_I, :], in_=G[:, SLOT_I, :],
                                     func=AF.Sigmoid)
            else:
                nc.scalar.activation(out=S[:, SLOT_F:SLOT_O + 1, :],
                                     in_=G[:, SLOT_F:SLOT_O + 1, :],
                                     func=AF.Sigmoid)
        else:
            b0 = (l - 1) * 4
            gates = [(SLOT_G, AF.Tanh), (SLOT_F, AF.Sigmoid),
                     (SLOT_I, AF.Sigmoid)]
            if not defer_o:
                gates.append((SLOT_O, AF.Sigmoid))
            for s, fn in gates:
                nc.scalar.activation(out=S[:, s, :], in_=G[:, s, :], func=fn,
                                     bias=br_sb[:, b0 + s:b0 + s + 1])

        if k == 0:
            c = sbS.tile([128, bk], F32, tag=f"c{cc}")
            c_st[cc] = c
            nc.vector.tensor_mul(c, S[:, SLOT_I, :], S[:, SLOT_G, :])  # i*g
        else:
            c = c_st[cc]
            nc.gpsimd.tensor_mul(c, S[:, SLOT_F, :], c)            # f*c (POOL)
            u = sbB.tile([128, bk], BF16, tag=f"u{cc}")
            nc.vector.tensor_mul(u, S[:, SLOT_I, :], S[:, SLOT_G, :])  # (DVE)
            nc.gpsimd.tensor_add(c, c, u)                          # (POOL)
        return S, c, (sig_o if defer_o else None)

    def lstm_step_ph2(cc, l, k, S, c, sig_o):
        if sig_o is not None:
            sig_o()
        # second phase emitted after the other chunk's phase 1 so the
        # ScalarE FIFO never head-of-line blocks on tanh(c) while the other
        # chunk's (ready) sigmoid sits behind it
        c0 = cc * bk
        cols = slice(c0, c0 + bk)
        rt = t_len - 1 - k
        Tc = sbB.tile([128, bk], BF16, tag=f"tc{cc}")
        nc.scalar.activation(out=Tc, in_=c, func=AF.Tanh)
        h = sbA.tile([128, bk], BF16, tag=f"h{cc}")
        nc.vector.tensor_mul(h, S[:, SLOT_O, :], Tc)               # h = o*tanh(c)
        h_prev[cc] = h

        # store time-ordered halves: fwd half at t=k, bwd half at t=rt
        nc.sync.dma_start(out=O[l][0:64, k, cols], in_=h[0:64, :])
        nc.sync.dma_start(out=O[l][64:128, rt, cols], in_=h[64:128, :])

    PF = min(2, t_len - 1)
    for l in range(NL):
        pend = {}
        for kk in range(PF):
            for cc in range(CHUNKS):
                pend[(cc, kk)] = issue_inp(cc, l, kk)
        for k in range(t_len):
            ph1 = {}
            for cc in range(CHUNKS):
                if k + PF < t_len:
                    pend[(cc, k + PF)] = issue_inp(cc, l, k + PF)
                ph1[cc] = lstm_step(cc, l, k, pend.pop((cc, k)))
            for cc in range(CHUNKS):
                S, c, sig_o = ph1[cc]
                lstm_step_ph2(cc, l, k, S, c, sig_o)

    # ---- final stage: relu + residual + LayerNorm + transpose to natural ----
    def issue_fin(cc, t):
        c0 = cc * bk
        cols = slice(c0, c0 + bk)
        o2t = sbA.tile([128, bk], BF16, tag=f"inf{cc}", bufs=4, name="o2t")
        nc.gpsimd.dma_start(out=o2t, in_=O[NL - 1][:, t, cols])
        xt = sbA.tile([9, bk], BF16, tag=f"inb{cc}", bufs=4, name="xt")
        nc.scalar.dma_start(out=xt, in_=xaug[0:9, t, cols])
        return o2t, xt

    fin_pend = [{} for _ in range(CHUNKS)]
    muvar2 = [None] * CHUNKS

    def final_t(cc, t, o2t, xt):
        c0 = cc * bk
        relu4 = sbB.tile([128, bk], F32, tag=f"relu{cc}")
        nc.vector.tensor_scalar_max(relu4, o2t, 0.0)
        # one accumulation group for the whole bank: transpose overwrites its
        # quarter (pending-zero from the single start), residual accumulates
        psZ = ps.tile([128, 4, 128], F32, tag=f"g{cc}")
        for bi in range(4):
            bs = slice(bi * 128, (bi + 1) * 128)
            nc.tensor.matmul(psZ[:, bi, :], relu4[:, bs], ident_sb,
                             is_transpose=True, start=(bi == 0), stop=False,
                             skip_group_check=True)
            nc.tensor.matmul(psZ[:, bi, :], xt[:, bs], wres_sb,
                             start=False, stop=(bi == 3),
                             skip_group_check=True)
        # z to SBUF once on the (otherwise idle) ScalarE, stats + normalize
        # read it from SBUF on DVE/Pool
        z = sbB.tile([128, 4, 128], F32, tag=f"z{cc}", bufs=3)
        nc.scalar.copy(out=z, in_=psZ)
        tp = t % 2
        if tp == 0:
            muvar2[cc] = sbB.tile([128, 8, 2], F32, tag=f"mv{cc}",
                                  name="muvar2")
        bnst = sbB.tile([128, 4, 6], F32, tag=f"bn{cc}")
        for bi in range(4):
            nc.vector.bn_stats(bnst[:, bi, :], z[:, bi, :])
            nc.vector.bn_aggr(muvar2[cc][:, tp * 4 + bi, :], bnst[:, bi, :])
        fin_pend[cc][tp] = (t, z)
        if tp == 0:
            return
        # shared sqrt/reciprocal/(-mu*rstd) pipeline for the t-pair
        mv = muvar2[cc]
        sd = sbB.tile([128, 8, 1], F32, tag=f"sd{cc}")
        nc.scalar.activation(out=sd, in_=mv[:, :, 1:2], func=AF.Sqrt,
                             bias=eps_sb)
        rstd = sbB.tile([128, 8, 1], F32, tag=f"rs{cc}")
        nc.vector.reciprocal_approx_fast(rstd, sd)
        nmr = sbB.tile([128, 8, 1], F32, tag=f"nm{cc}")
        nc.vector.scalar_tensor_tensor(nmr, mv[:, :, 0:1], -1.0, rstd,
                                       op0=OP.mult, op1=OP.mult)
        for tq in (0, 1):
            tt, zq = fin_pend[cc].pop(tq)
            onat = sbA.tile([128, 4, 128], F32, tag=f"on{cc}", bufs=3)
            for bi in range(4):
                qb = tq * 4 + bi
                nc.gpsimd.tensor_scalar(onat[:, bi, :], zq[:, bi, :],
                                        rstd[:, qb, :], nmr[:, qb, :],
                                        op0=OP.mult, op1=OP.add)
                if apply_gb:
                    nc.gpsimd.tensor_mul(onat[:, bi, :], onat[:, bi, :],
                                         gamma_sb)
                    nc.gpsimd.tensor_add(onat[:, bi, :], onat[:, bi, :],
                                         beta_sb)
            oap = out_d[c0:c0 + bk, tt, :].rearrange("(b p) f -> p b f",
                                                     p=128)
            nc.sync.dma_start(out=oap, in_=onat)

    fpend = {}
    for tt in range(PF):
        for cc in range(CHUNKS):
            fpend[(cc, tt)] = issue_fin(cc, tt)
    for t in range(t_len):
        for cc in range(CHUNKS):
            if t + PF < t_len:
                fpend[(cc, t + PF)] = issue_fin(cc, t + PF)
            o2t, xt = fpend.pop((cc, t))
            final_t(cc, t, o2t, xt)


def build(apply_gb=False, bc=BC, t_len=T, num_devices=NCORES):
    nc = bacc.Bacc("TRN2", target_bir_lowering=False, debug=False,
                   num_devices=num_devices)
    D = {}

    def inp(name, shape, dtype=F32):
        D[name] = nc.dram_tensor(name, shape, dtype, kind="ExternalInput").ap()

    inp("xaug", [18, t_len, bc], BF16)
    inp("rw", [128, NL, 4, 128], BF16)
    inp("pw", [128, NL - 1, 4, 2, 64], BF16)
    inp("l0w", [18, 4, 128], BF16)
    inp("br", [128, (NL - 1) * 4])
    inp("wres", [9, 128], BF16)
    inp("ident", [128, 128])
    if apply_gb:
        inp("gammab", [128, 128])
        inp("betab", [128, 128])
    for i in range(NL):
        D[f"o{i}"] = nc.dram_tensor(f"o{i}", [128, t_len, bc], BF16).ap()
    D["out"] = nc.dram_tensor("out", [bc, t_len, 128], F32,
                              kind="ExternalOutput").ap()

    with tile.TileContext(nc) as tc:
        with ExitStack() as ctx:
            _emit(nc, tc, ctx, D, apply_gb, bc, t_len)
    nc.compile()
    return nc


_BUILD_CACHE = {}


def make_in_maps(inputs, apply_gb):
    ln_gamma = np.asarray(inputs["ln_gamma"], np.float32)
    ln_beta = np.asarray(inputs["ln_beta"], np.float32)
    shared, xaug_cores = _host_prep(
        inputs["x"], inputs["w_ih"], inputs["w_hh"], inputs["b_ih"],
        inputs["b_hh"], inputs["w_res"], inputs["b_res"], NCORES, BC)
    in_maps = []
    for c in range(NCORES):
        m = dict(shared)
        m["xaug"] = xaug_cores[c]
        if apply_gb:
            m["gammab"] = np.ascontiguousarray(
                np.broadcast_to(ln_gamma, (128, 128)).astype(np.float32))
            m["betab"] = np.ascontiguousarray(
                np.broadcast_to(ln_beta, (128, 128)).astype(np.float32))
        in_maps.append(m)
    return in_maps


def kernel(x, w_ih, w_hh, b_ih, b_hh, w_res, b_res, ln_gamma, ln_beta):
    ln_gamma = np.asarray(ln_gamma, np.float32)
    ln_beta = np.asarray(ln_beta, np.float32)
    apply_gb = not (np.all(ln_gamma == 1.0) and np.all(ln_beta == 0.0))

    if apply_gb not in _BUILD_CACHE:
        _BUILD_CACHE[apply_gb] = build(apply_gb)
    nc = _BUILD_CACHE[apply_gb]

    inputs = dict(x=x, w_ih=w_ih, w_hh=w_hh, b_ih=b_ih, b_hh=b_hh,
                  w_res=w_res, b_res=b_res, ln_gamma=ln_gamma, ln_beta=ln_beta)
    in_maps = make_in_maps(inputs, apply_gb)

    res = run_bass_kernel_spmd(nc, in_maps, core_ids=list(range(NCORES)))
    out = np.concatenate([res.results[c]["out"] for c in range(NCORES)], axis=0)
    return np.ascontiguousarray(out.astype(np.float32))


# revision 27
# speedup vs baseline: 2.0741x; 1.0094x over previous
"""Trainium2 Bass kernel for a 3-layer BiLSTM + ReLU + residual + LayerNorm.

Strategy (pure data parallel over 8 cores, 1024 batch rows per core):
  * "Transposed" layout on-chip: features on SBUF partitions, batch on the
    free dim.  Both directions fused on partitions (fwd = 0:64, bwd = 64:128)
    so every engine op runs with all 128 lanes busy.
  * All matmuls in bf16 (4x cheaper per row than fp32 on the PE): per
    timestep, per gate, a col-tiled input-projection pair plus one 128x128
    block-diagonal recurrent matmul accumulate into one PSUM bank; the four
    gates share a [128, 4, bk] PSUM tensor ordered (i, f, o, g).
  * Layer 0 folds both time directions AND the biases into a single K=18
    augmented input ([x_t; 1; x_rt; 1]), so its activations need no bias:
    one sigmoid over the (i,f,o) span + one tanh for g.
  * Layers 1-2 keep per-gate activations with per-partition bias.
  * Elementwise rebalance: i*g and h=o*tanh(c) on DVE in bf16, f*c on the
    Pool engine, c accumulates in fp32 on DVE.
  * Final stage: bf16 PE transpose to natural layout + K=9 residual matmul
    into one PSUM bank, LayerNorm stats via one bn_stats + four bn_aggr on
    DVE, rstd via the approx reciprocal, normalization on Pool directly from
    PSUM, DMA out natural-layout fp32.
  * Hot-loop input DMAs issue from the Pool queue (cheap dispatch), output
    stores from the SP queue.
"""

from contextlib import ExitStack

import numpy as np
import ml_dtypes

import concourse.bacc as bacc
import concourse.tile as tile
from concourse import mybir
from concourse.bass_utils import run_bass_kernel_spmd

F32 = mybir.dt.float32
BF16 = mybir.dt.bfloat16
AF = mybir.ActivationFunctionType
OP = mybir.AluOpType

NCORES = 8
BC = 1024               # batch rows per core
CHUNKS = 2
T = 64
H = 64
NL = 3
D2 = 2 * H              # 128
LN_EPS = 1e-5

# gate order in PyTorch weights: i, f, g, o (rows g*H:(g+1)*H of w_ih/w_hh)
# on-chip slot order: g, f, i, o -- g first so its tanh clears the ACT queue
# early, f second so the Pool f*c starts before the i/o sigmoids; i,o
# adjacent so layer-0 can sigmoid them as one span
PT2SLOT = {0: 2, 1: 1, 2: 0, 3: 3}   # pytorch gate idx -> PSUM slot
SLOT_G, SLOT_F, SLOT_I, SLOT_O = 0, 1, 2, 3


def _host_prep(x, w_ih, w_hh, b_ih, b_hh, w_res, b_res, ncores, bc):
    """Matmul-ready bf16 weight layouts (shared across cores) + per-core
    inputs."""
    x = np.asarray(x, np.float32)
    w_ih = np.asarray(w_ih, np.float32)
    w_hh = np.asarray(w_hh, np.float32)
    bias = np.asarray(b_ih, np.float32) + np.asarray(b_hh, np.float32)  # (NL,2,4H)
    w_res = np.asarray(w_res, np.float32)
    b_res = np.asarray(b_res, np.float32)
    t_len = x.shape[1]

    # Recurrent lhsT, K-major: rw[k, l, slot, m] (block-diagonal over dirs)
    rw = np.zeros((128, NL, 4, 128), np.float32)
    for l in range(NL):
        for g in range(4):
            s = PT2SLOT[g]
            gs = slice(g * H, (g + 1) * H)
            rw[0:64, l, s, 0:64] = w_hh[l, 0, gs, :].T
            rw[64:128, l, s, 64:128] = w_hh[l, 1, gs, :].T
    rw = rw.astype(ml_dtypes.bfloat16)

    # Input-projection lhsT for layers 1,2 (bf16): pw[k, l-1, slot, d, m]
    pw = np.zeros((128, NL - 1, 4, 2, 64), np.float32)
    for l in (1, 2):
        for g in range(4):
            s = PT2SLOT[g]
            gs = slice(g * H, (g + 1) * H)
            for d in range(2):
                pw[:, l - 1, s, d, :] = w_ih[l, d, gs, :].T
    pw = pw.astype(ml_dtypes.bfloat16)

    # Layer-0 fused fwd+bwd projection lhsT with biases on the ones-rows:
    # K rows 0:8 = x_t, row 8 = 1, rows 9:17 = x_rt, row 17 = 1.
    l0w = np.zeros((18, 4, 128), np.float32)
    for g in range(4):
        s = PT2SLOT[g]
        gs = slice(g * H, (g + 1) * H)
        l0w[0:8, s, 0:64] = w_ih[0, 0, gs, 0:8].T
        l0w[8, s, 0:64] = bias[0, 0, gs]
        l0w[9:17, s, 64:128] = w_ih[0, 1, gs, 0:8].T
        l0w[17, s, 64:128] = bias[0, 1, gs]
    l0w = l0w.astype(ml_dtypes.bfloat16)

    # per-partition gate biases, layers 1,2 (fused dirs): br[p, (l-1)*4+slot]
    br = np.zeros((128, (NL - 1) * 4), np.float32)
    for l in (1, 2):
        for g in range(4):
            s = PT2SLOT[g]
            gs = slice(g * H, (g + 1) * H)
            br[0:64, (l - 1) * 4 + s] = bias[l, 0, gs]
            br[64:128, (l - 1) * 4 + s] = bias[l, 1, gs]

    # residual rhs: wres[k, f] = w_res[f, k], row 8 = b_res
    wres = np.zeros((9, 128), np.float32)
    wres[0:8, :] = w_res.T
    wres[8, :] = b_res
    wres = wres.astype(ml_dtypes.bfloat16)

    ident = np.eye(128, dtype=np.float32)

    # Per-core transposed-augmented input: xaug[k, t, b] with both time
    # directions stacked: rows 0:8 = x_t, 8 = 1, 9:17 = x_{T-1-t}, 17 = 1
    xaug_cores = []
    for c in range(ncores):
        xc = x[c * bc:(c + 1) * bc]              # (bc, T, 8)
        xa = np.empty((18, t_len, bc), np.float32)
        xa[0:8] = xc.transpose(2, 1, 0)
        xa[8] = 1.0
        xa[9:17] = xc[:, ::-1].transpose(2, 1, 0)
        xa[17] = 1.0
        xaug_cores.append(xa.astype(ml_dtypes.bfloat16))

    shared = dict(rw=rw, pw=pw, l0w=l0w, br=br, wres=wres, ident=ident)
    return shared, xaug_cores


def _emit(nc, tc, ctx, D, apply_gb, bc, t_len):
    bk = bc // CHUNKS

    sbC = ctx.enter_context(tc.tile_pool(name="consts", bufs=1))
    sbA = ctx.enter_context(tc.tile_pool(name="workA", bufs=3))
    sbB = ctx.enter_context(tc.tile_pool(name="workB", bufs=2))
    sbS = ctx.enter_context(tc.tile_pool(name="state", bufs=1))
    ps = ctx.enter_context(tc.tile_pool(name="ps", bufs=1, space="PSUM"))

    def const_tile(shape, dtype, key):
        t = sbC.tile(shape, dtype, name=f"c_{key}", tag=f"c_{key}")
        nc.sync.dma_start(out=t, in_=D[key])
        return t

    rw_sb = const_tile([128, NL, 4, 128], BF16, "rw")
    pw_sb = const_tile([128, NL - 1, 4, 2, 64], BF16, "pw")
    l0w_sb = const_tile([18, 4, 128], BF16, "l0w")
    br_sb = const_tile([128, (NL - 1) * 4], F32, "br")
    wres_sb = const_tile([9, 128], BF16, "wres")
    ident_sb = const_tile([128, 128], F32, "ident")
    gamma_sb = beta_sb = None
    if apply_gb:
        gamma_sb = const_tile([128, 128], F32, "gammab")
        beta_sb = const_tile([128, 128], F32, "betab")
    eps_sb = sbC.tile([128, 1], F32)
    nc.vector.memset(eps_sb, LN_EPS)

    O = [D[f"o{i}"] for i in range(NL)]
    xaug = D["xaug"]
    out_d = D["out"]

    h_prev = [None] * CHUNKS
    c_st = [None] * CHUNKS

    def issue_inp(cc, l, k):
        c0 = cc * bk
        cols = slice(c0, c0 + bk)
        rt = t_len - 1 - k
        if l == 0:
            xin = sbA.tile([18, bk], BF16, tag=f"inf{cc}", bufs=4, name="xin")
            nc.gpsimd.dma_start(out=xin, in_=xaug[:, k, cols])
            return (xin,)
        # both time slices (k and rt) in one strided DMA; half order follows
        # the slice direction
        t0, t1 = (k, rt) if k < rt else (rt, k)
        inp2 = sbA.tile([128, 2, bk], BF16, tag=f"inf{cc}", bufs=4,
                        name="inp2")
        nc.gpsimd.dma_start(out=inp2,
                            in_=O[l - 1][:, t0:t1 + 1:t1 - t0, cols])
        if k < rt:
            return (inp2[:, 0, :], inp2[:, 1, :])
        return (inp2[:, 1, :], inp2[:, 0, :])

    def lstm_step(cc, l, k, inps):
        G = ps.tile([128, 4, bk], F32, tag=f"g{cc}")

        for s in range(4):
            if l == 0:
                (xin,) = inps
                # fused fwd+bwd proj with bias rows, K=18
                nc.tensor.matmul(G[:, s, :], l0w_sb[:, s, :], xin,
                                 start=True, stop=(k == 0),
                                 skip_group_check=True)
            else:
                inp_f, inp_b = inps
                calls = [
                    (G[0:64, s, :], pw_sb[:, l - 1, s, 0, :], inp_f,
                     (0, 0), (0, 64)),
                    (G[64:128, s, :], pw_sb[:, l - 1, s, 1, :], inp_b,
                     (0, 64), (64, 128)),
                ]
                if k > 0:
                    calls.append((G[:, s, :], rw_sb[:, l, s, :], h_prev[cc],
                                  None, (0, 128)))
                n = len(calls)
                for i, (o, lh, rh, tp, rng) in enumerate(calls):
                    covered = set()
                    for _, _, _, _, r in calls[:i]:
                        covered.update(range(*r))
                    start = not set(range(*rng)).issubset(covered)
                    stop = not any(max(rng[0], r[0]) < min(rng[1], r[1])
                                   for _, _, _, _, r in calls[i + 1:])
                    # skip_group_check: the executing-sim group checker
                    # mis-addresses partition-based PSUM offsets (tensor rows
                    # != 16KB); data semantics are still simulated exactly.
                    nc.tensor.matmul(o, lh, rh, start=start, stop=stop,
                                     tile_position=tp, skip_group_check=True)
            if l == 0 and k > 0:
                nc.tensor.matmul(G[:, s, :], rw_sb[:, 0, s, :], h_prev[cc],
                                 start=False, stop=True,
                                 skip_group_check=True)

        # the o sigmoid is off the c-critical-path; deferring it for the
        # second chunk lets the first chunk's tanh(c) enter the ACT queue
        # earlier, hiding the c-chain and h->rec latencies of both chunks
        defer_o = (cc == 1)
        S = sbB.tile([128, 4, bk], BF16, tag=f"s{cc}", bufs=3)

        def sig_o():
            if l == 0:
                nc.scalar.activation(out=S[:, SLOT_O, :], in_=G[:, SLOT_O, :],
                                     func=AF.Sigmoid)
            else:
                b0 = (l - 1) * 4
                nc.scalar.activation(out=S[:, SLOT_O, :], in_=G[:, SLOT_O, :],
                                     func=AF.Sigmoid,
                                     bias=br_sb[:, b0 + SLOT_O:b0 + SLOT_O + 1])

        if l == 0:
            # biases folded into the projection: bias-free activations
            nc.scalar.activation(out=S[:, SLOT_G, :], in_=G[:, SLOT_G, :],
                                 func=AF.Tanh)
            if defer_o:
                # f separate so f*c starts early; o deferred to ph2
                nc.scalar.activation(out=S[:, SLOT_F, :], in_=G[:, SLOT_F, :],
                                     func=AF.Sigmoid)
                nc.scalar.activation(out=S[:, SLOT_I, :], in_=G[:, SLOT_I, :],
                                     func=AF.Sigmoid)
            else:
                nc.scalar.activation(out=S[:, SLOT_F:SLOT_O + 1, :],
                                     in_=G[:, SLOT_F:SLOT_O + 1, :],
                                     func=AF.Sigmoid)
        else:
            b0 = (l - 1) * 4
            gates = [(SLOT_G, AF.Tanh), (SLOT_F, AF.Sigmoid),
                     (SLOT_I, AF.Sigmoid)]
            if not defer_o:
                gates.append((SLOT_O, AF.Sigmoid))
            for s, fn in gates:
                nc.scalar.activation(out=S[:, s, :], in_=G[:, s, :], func=fn,
                                     bias=br_sb[:, b0 + s:b0 + s + 1])

        if k == 0:
            c = sbS.tile([128, bk], F32, tag=f"c{cc}")
            c_st[cc] = c
            nc.vector.tensor_mul(c, S[:, SLOT_I, :], S[:, SLOT_G, :])  # i*g
        else:
            c = c_st[cc]
            nc.gpsimd.tensor_mul(c, S[:, SLOT_F, :], c)            # f*c (POOL)
            u = sbB.tile([128, bk], BF16, tag=f"u{cc}")
            nc.vector.tensor_mul(u, S[:, SLOT_I, :], S[:, SLOT_G, :])  # (DVE)
            nc.gpsimd.tensor_add(c, c, u)                          # (POOL)
        return S, c, (sig_o if defer_o else None)

    def lstm_step_ph2(cc, l, k, S, c, sig_o):
        if sig_o is not None:
            sig_o()
        # second phase emitted after the other chunk's phase 1 so the
        # ScalarE FIFO never head-of-line blocks on tanh(c) while the other
        # chunk's (ready) sigmoid sits behind it
        c0 = cc * bk
        cols = slice(c0, c0 + bk)
        rt = t_len - 1 - k
        Tc = sbB.tile([128, bk], BF16, tag=f"tc{cc}")
        nc.scalar.activation(out=Tc, in_=c, func=AF.Tanh)
        h = sbA.tile([128, bk], BF16, tag=f"h{cc}")
        nc.vector.tensor_mul(h, S[:, SLOT_O, :], Tc)               # h = o*tanh(c)
        h_prev[cc] = h

        # store time-ordered halves: fwd half at t=k, bwd half at t=rt
        nc.sync.dma_start(out=O[l][0:64, k, cols], in_=h[0:64, :])
        nc.sync.dma_start(out=O[l][64:128, rt, cols], in_=h[64:128, :])

    PF = min(2, t_len - 1)
    for l in range(NL):
        pend = {}
        for kk in range(PF):
            for cc in range(CHUNKS):
                pend[(cc, kk)] = issue_inp(cc, l, kk)
        for k in range(t_len):
            ph1 = {}
            for cc in range(CHUNKS):
                if k + PF < t_len:
                    pend[(cc, k + PF)] = issue_inp(cc, l, k + PF)
                ph1[cc] = lstm_step(cc, l, k, pend.pop((cc, k)))
            for cc in range(CHUNKS):
                S, c, sig_o = ph1[cc]
                lstm_step_ph2(cc, l, k, S, c, sig_o)

    # ---- final stage: relu + residual + LayerNorm + transpose to natural ----
    def issue_fin(t):
        o2t = sbA.tile([128, bc], BF16, tag="inf0", bufs=4, name="o2t")
        nc.gpsimd.dma_start(out=o2t, in_=O[NL - 1][:, t, :])
        xt = sbA.tile([9, bc], BF16, tag="inb0", bufs=4, name="xt")
        nc.scalar.dma_start(out=xt, in_=xaug[0:9, t, :])
        return o2t, xt

    fin_pend = {}
    muvar2 = [None]

    def final_t(t, o2t, xt):
        relu4 = sbB.tile([128, bc], F32, tag="relu0")
        nc.vector.tensor_scalar_max(relu4, o2t, 0.0)
        # per PSUM bank: transpose overwrites its quarter (pending-zero from
        # the bank's single start), residual accumulates; alternate the two
        # LSTM psum tags for double buffering across t
        psZ = ps.tile([128, 8, 128], F32, tag=f"g{t % 2}", name="psZ")
        for bi in range(8):
            bs = slice(bi * 128, (bi + 1) * 128)
            nc.tensor.matmul(psZ[:, bi, :], relu4[:, bs], ident_sb,
                             is_transpose=True, start=(bi % 4 == 0),
                             stop=False, skip_group_check=True)
            nc.tensor.matmul(psZ[:, bi, :], xt[:, bs], wres_sb,
                             start=False, stop=(bi % 4 == 3),
                             skip_group_check=True)
        # z to SBUF once on the (otherwise idle) ScalarE, stats + normalize
        # read it from SBUF on DVE/Pool
        z = sbB.tile([128, 8, 128], F32, tag="z0", bufs=3)
        nc.scalar.copy(out=z, in_=psZ)
        tp = t % 2
        if tp == 0:
            muvar2[0] = sbB.tile([128, 16, 2], F32, tag="mv0", name="muvar2")
        bnst = sbB.tile([128, 8, 6], F32, tag="bn0")
        for bi in range(8):
            nc.vector.bn_stats(bnst[:, bi, :], z[:, bi, :])
            nc.vector.bn_aggr(muvar2[0][:, tp * 8 + bi, :], bnst[:, bi, :])
        fin_pend[tp] = (t, z)
        if tp == 0:
            return
        # shared sqrt/reciprocal/(-mu*rstd) pipeline for the t-pair
        mv = muvar2[0]
        sd = sbB.tile([128, 16, 1], F32, tag="sd0")
        nc.scalar.activation(out=sd, in_=mv[:, :, 1:2], func=AF.Sqrt,
                             bias=eps_sb)
        rstd = sbB.tile([128, 16, 1], F32, tag="rs0")
        nc.vector.reciprocal_approx_fast(rstd, sd)
        nmr = sbB.tile([128, 16, 1], F32, tag="nm0")
        nc.vector.scalar_tensor_tensor(nmr, mv[:, :, 0:1], -1.0, rstd,
                                       op0=OP.mult, op1=OP.mult)
        for tq in (0, 1):
            tt, zq = fin_pend.pop(tq)
            onat = sbA.tile([128, 8, 128], F32, tag="on0", bufs=3)
            for bi in range(8):
                qb = tq * 8 + bi
                nc.gpsimd.tensor_scalar(onat[:, bi, :], zq[:, bi, :],
                                        rstd[:, qb, :], nmr[:, qb, :],
                                        op0=OP.mult, op1=OP.add)
                if apply_gb:
                    nc.gpsimd.tensor_mul(onat[:, bi, :], onat[:, bi, :],
                                         gamma_sb)
                    nc.gpsimd.tensor_add(onat[:, bi, :], onat[:, bi, :],
                                         beta_sb)
            oap = out_d[:, tt, :].rearrange("(b p) f -> p b f", p=128)
            nc.sync.dma_start(out=oap, in_=onat)

    fpend = {}
    for tt in range(PF):
        fpend[tt] = issue_fin(tt)
    for t in range(t_len):
        if t + PF < t_len:
            fpend[t + PF] = issue_fin(t + PF)
        o2t, xt = fpend.pop(t)
        final_t(t, o2t, xt)


def build(apply_gb=False, bc=BC, t_len=T, num_devices=NCORES):
    nc = bacc.Bacc("TRN2", target_bir_lowering=False, debug=False,
                   num_devices=num_devices)
    D = {}

    def inp(name, shape, dtype=F32):
        D[name] = nc.dram_tensor(name, shape, dtype, kind="ExternalInput").ap()

    inp("xaug", [18, t_len, bc], BF16)
    inp("rw", [128, NL, 4, 128], BF16)
    inp("pw", [128, NL - 1, 4, 2, 64], BF16)
    inp("l0w", [18, 4, 128], BF16)
    inp("br", [128, (NL - 1) * 4])
    inp("wres", [9, 128], BF16)
    inp("ident", [128, 128])
    if apply_gb:
        inp("gammab", [128, 128])
        inp("betab", [128, 128])
    for i in range(NL):
        D[f"o{i}"] = nc.dram_tensor(f"o{i}", [128, t_len, bc], BF16).ap()
    D["out"] = nc.dram_tensor("out", [bc, t_len, 128], F32,
                              kind="ExternalOutput").ap()

    with tile.TileContext(nc) as tc:
        with ExitStack() as ctx:
            _emit(nc, tc, ctx, D, apply_gb, bc, t_len)
    nc.compile()
    return nc


_BUILD_CACHE = {}


def make_in_maps(inputs, apply_gb):
    ln_gamma = np.asarray(inputs["ln_gamma"], np.float32)
    ln_beta = np.asarray(inputs["ln_beta"], np.float32)
    shared, xaug_cores = _host_prep(
        inputs["x"], inputs["w_ih"], inputs["w_hh"], inputs["b_ih"],
        inputs["b_hh"], inputs["w_res"], inputs["b_res"], NCORES, BC)
    in_maps = []
    for c in range(NCORES):
        m = dict(shared)
        m["xaug"] = xaug_cores[c]
        if apply_gb:
            m["gammab"] = np.ascontiguousarray(
                np.broadcast_to(ln_gamma, (128, 128)).astype(np.float32))
            m["betab"] = np.ascontiguousarray(
                np.broadcast_to(ln_beta, (128, 128)).astype(np.float32))
        in_maps.append(m)
    return in_maps


def kernel(x, w_ih, w_hh, b_ih, b_hh, w_res, b_res, ln_gamma, ln_beta):
    ln_gamma = np.asarray(ln_gamma, np.float32)
    ln_beta = np.asarray(ln_beta, np.float32)
    apply_gb = not (np.all(ln_gamma == 1.0) and np.all(ln_beta == 0.0))

    if apply_gb not in _BUILD_CACHE:
        _BUILD_CACHE[apply_gb] = build(apply_gb)
    nc = _BUILD_CACHE[apply_gb]

    inputs = dict(x=x, w_ih=w_ih, w_hh=w_hh, b_ih=b_ih, b_hh=b_hh,
                  w_res=w_res, b_res=b_res, ln_gamma=ln_gamma, ln_beta=ln_beta)
    in_maps = make_in_maps(inputs, apply_gb)

    res = run_bass_kernel_spmd(nc, in_maps, core_ids=list(range(NCORES)))
    out = np.concatenate([res.results[c]["out"] for c in range(NCORES)], axis=0)
    return np.ascontiguousarray(out.astype(np.float32))
